# revision 1
# baseline (speedup 1.0000x reference)
"""HardAndLayer on 8 Trainium2 NeuronCores.

out[l] = AND_d (x[d] OR NOT w[l,d])  ==  no d with (w[l,d] AND NOT x[d])

Strategy (per sharding hint): shard bit_weights row-wise (neuron dim) across
8 cores, x replicated, no collectives.

Wire format: the bool tensors are bit-packed on the host, 31 bools per
32-bit word with bit 30 (top fp32 exponent bit) forced to zero, so no word
can form a NaN/Inf pattern. Each core moves ~1.2 MB instead of 8 MB over
HBM. On device a custom fused DVE op computes, per neuron row,
    acc[p] = fold_logical_or_j (w_packed[p, j] BITWISE_AND notx_packed[j])
in a single pass: the streams are declared fp32 (identity converter — no
int conversion), BITWISE_AND preserves raw bits, and LOGICAL_OR folds on
bit-pattern truthiness (HW-verified: -0.0-only words count as violations).
out[l] = (acc == 0), applied on the host to the DMA'd per-neuron flags.
All reduction math happens on device; host packing/relabeling is layout
only.

Layout: partition p of a core holds its 8 consecutive neuron rows
(8 KB contiguous per partition) so the weight shard arrives in a few large
DMAs, and res[p, b] = out[8p + b] is identity-ordered on the host.
"""

import numpy as np

L = 8192
D = 8192
NCORES = 8
LSH = L // NCORES  # 1024 neuron rows per core
PAYLOAD = 31  # bits per packed word (bit 30 held zero -> never NaN/Inf)
WPK = -(-D // PAYLOAD)  # 265 packed words per neuron row
DPAD = WPK * PAYLOAD
# payload bit positions: 0..29 and 31 (skip bit 30)
_BITPOS = list(range(30)) + [31]
NB = LSH // 128  # 8 neuron rows per partition
# Per-partition DRAM layout: [notx | row0 | ... | row7], 9*WPK words
# contiguous per partition. Chunks in row-units (chunk 0 carries notx).
CHUNK_UNITS = (3, 2, 2, 1, 1)
CHUNK_COLS = tuple(u * WPK for u in CHUNK_UNITS)

_compiled = None
_custom_op = None


def _register_custom_op():
    """Register the fused AND+any op in the custom-DVE table (idempotent)."""
    global _custom_op
    if _custom_op is not None:
        return _custom_op
    from concourse import dve_ops
    from concourse.dve_spec import Spec, Src0, Src1, Zero, Bin, lower
    from concourse.dve_uop import AluOp, DveOpSpec

    name = "AND_ANY_ANT"
    for o in dve_ops.OPS:
        if o.name == name:
            _custom_op = o
            return o

    def _ref(in0, in1, c0, c1, c2):
        a = in0.view(np.uint32) & in1.view(np.uint32)
        acc = (
            (a.reshape(a.shape[0], -1) != 0)
            .any(axis=-1, keepdims=True)
            .astype(np.float32)
        )
        return a.view(np.float32), acc

    spec = Spec(
        body=Bin(AluOp.BITWISE_AND, Src0, Src1),
        accum=AluOp.LOGICAL_OR,
        accum_init=Zero,
        reference=_ref,
    )
    shas = {}
    for ver in ("v3", "v4"):
        try:
            uops = lower(spec, ver=ver)
            shas[ver] = DveOpSpec(name=name, uops=uops, rd1_en=True).sha(ver)
        except Exception:
            pass
    op = dve_ops.DveOp(name, spec, subdim=False, uops_sha=shas)
    dve_ops.OPS.append(op)
    dve_ops._SUB_OPCODE_FOR_NAME[name] = (
        dve_ops._CUSTOM_DVE_ROW_BASE + len(dve_ops.OPS) - 1
    )
    dve_ops.CUSTOM_DVE_SPECS[name] = spec
    _custom_op = op
    return op


def _build():
    import concourse.bacc as bacc
    import concourse.mybir as mybir
    from concourse import tile

    op = _register_custom_op()

    nc = bacc.Bacc(
        "TRN2",
        target_bir_lowering=False,
        debug=False,
        enable_asserts=False,
        num_devices=NCORES,
    )
    TOT = (NB + 1) * WPK
    wx = nc.dram_tensor("wx", [128, TOT], mybir.dt.float32, kind="ExternalInput")
    res = nc.dram_tensor("res", [128, NB], mybir.dt.float32, kind="ExternalOutput")

    with tile.TileContext(nc) as tc:
        with (
            tc.tile_pool(name="wpool", bufs=1) as wpool,
            tc.tile_pool(name="mpool", bufs=2) as mpool,
            tc.tile_pool(name="small", bufs=1) as small,
        ):
            acc = small.tile([128, NB], mybir.dt.float32)
            tiles = []
            c0 = 0
            for ci, cw in enumerate(CHUNK_COLS):
                wt = wpool.tile([128, cw], mybir.dt.float32, tag=f"wt{ci}")
                dma_eng = nc.sync if ci % 2 == 0 else nc.scalar
                dma_eng.dma_start(wt[:], wx[:, c0 : c0 + cw])
                tiles.append((wt, c0, cw))
                c0 += cw
            nx_ap = tiles[0][0][:, 0:WPK]  # notx lives in chunk 0, col 0
            for gb in range(NB):
                col = (gb + 1) * WPK  # global word offset of neuron row gb
                for wt, tc0, tcw in tiles:
                    if tc0 <= col < tc0 + tcw:
                        in0 = wt[:, col - tc0 : col - tc0 + WPK]
                        break
                m = mpool.tile([128, WPK], mybir.dt.float32, tag="m")
                nc.vector._custom_dve(
                    op,
                    out=m[:],
                    in0=in0,
                    in1=nx_ap,
                    accum_out=acc[:, gb : gb + 1],
                )
            nc.sync.dma_start(res[:, :], acc[:])

    nc.compile()
    return nc


def _pack31(bits):
    """bits [..., D] uint8 -> [..., WPK] float32-viewed words, 31 bits/word
    at positions 0..29 and 31 (bit 30 always zero -> never NaN/Inf)."""
    lead = bits.shape[:-1]
    b32 = np.zeros(lead + (WPK, 32), dtype=np.uint8)
    pad = np.zeros(lead + (DPAD,), dtype=np.uint8)
    pad[..., :D] = bits
    pad = pad.reshape(lead + (WPK, PAYLOAD))
    b32[..., :30] = pad[..., :30]
    b32[..., 31] = pad[..., 30]
    words = np.packbits(b32.reshape(lead + (WPK * 32,)), axis=-1, bitorder="little")
    return words.view(np.uint32).view(np.float32)


def _pack_inputs(x, bit_weights):
    x = np.asarray(x).astype(np.uint8)
    bw = np.ascontiguousarray(np.asarray(bit_weights).astype(np.uint8))
    notx = (1 - x).astype(np.uint8)
    nxp = _pack31(notx)  # [WPK]
    wp = _pack31(bw)  # [L, WPK]
    in_maps = []
    for i in range(NCORES):
        shard = wp[i * LSH : (i + 1) * LSH].reshape(128, NB, WPK)
        wx = np.empty((128, NB + 1, WPK), dtype=np.float32)
        wx[:, 0, :] = nxp
        wx[:, 1:, :] = shard
        in_maps.append({"wx": wx.reshape(128, (NB + 1) * WPK)})
    return in_maps


def _gather(results):
    outs = []
    for i in range(NCORES):
        # [128, NB] fp32 violation flags; res[p, b] covers neuron 8p + b,
        # flag == 0.0 means no violated requirement -> output True
        res = results[i]["res"]
        outs.append(res.reshape(-1) == 0.0)
    return np.concatenate(outs).astype(np.bool_)


def _get_compiled():
    global _compiled
    if _compiled is None:
        _compiled = _build()
    return _compiled


def kernel(x, bit_weights):
    from concourse import bass_utils

    nc = _get_compiled()
    in_maps = _pack_inputs(x, bit_weights)
    r = bass_utils.run_bass_kernel_spmd(nc, in_maps, core_ids=list(range(NCORES)))
    return _gather(r.results)



# revision 30
# speedup vs baseline: 1.1747x; 1.1747x over previous
"""HardAndLayer on 8 Trainium2 NeuronCores.

out[l] = AND_d (x[d] OR NOT w[l,d])  ==  no d with (w[l,d] AND NOT x[d])

Strategy (per sharding hint): shard bit_weights row-wise (neuron dim) across
8 cores, x replicated, no collectives.

Wire format: bools are bit-packed 31 per int32 word with bit 30 forced
zero, so no word can form an fp32 NaN/Inf pattern (the DVE fp32 stream
path canonicalizes NaN operands — HW-verified failure with full 32-bit
packing). DRAM tensors are declared int32; on device the SBUF APs are
bitcast to fp32 for the custom DVE op, whose datapath is bitwise: per row
    acc[p] = fold_logical_or_j (w_packed[p, j] BITWISE_AND notx_packed[j])
(identity fp32 converter, BITWISE_AND preserves raw bits, LOGICAL_OR folds
on bit-pattern truthiness; -0.0/NaN/denormal patterns all count as nonzero).
out[l] = (acc == 0), applied on the host to the DMA'd per-neuron flags.

Pipeline: the weight shard streams in 6 HWDGE DMA chunks of decreasing
size (each chunk->DVE edge pays the ~900ns DMA-semaphore latency, so late
chunks are small to shorten the post-arrival compute tail). The result
write is a prepared SWDGE kv_writeback fired by trigger_dma right after
the last DVE accumulator write: its descriptors are generated early, off
the critical path, so the tail skips HWDGE descriptor-gen and the DGE
start delay entirely.

Layout: partition p of a core holds its 8 consecutive neuron rows
(8 KB contiguous per partition); res[p, b] = out[8p + b].
"""

import numpy as np

L = 8192
D = 8192
NCORES = 8
LSH = L // NCORES  # 1024 neuron rows per core
PAYLOAD = 31  # bits per packed word; bit 30 held zero -> never NaN/Inf.
# The DVE fp32 stream path canonicalizes NaN-pattern operands (HW-verified:
# 32-bit packing fails exactly on words with the exponent field all-ones),
# so the wire format must never form one. Denormals and -0.0 pass bit-exact.
WPK = -(-D // PAYLOAD)  # 265 packed words per neuron row
DPAD = WPK * PAYLOAD
_BITPOS = list(range(30)) + [31]  # payload bit positions (skip bit 30)
NB = LSH // 128  # 8 neuron rows per partition
# Per-partition DRAM layout: [notx | row0 | ... | row7], 9*WPK words.
# Chunk sizes in row units (1 unit = WPK words = 1 KiB/partition):
# (notx,r0 | r1,r2 | r3,r4 | r5,r6 | r7) -- small first chunk starts DVE
# early; 5 chunks keep the serial HWDGE descriptor-gen off the DMA stream.
CHUNK_UNITS = (2, 2, 2, 2, 1)
assert sum(CHUNK_UNITS) == NB + 1
CHUNK_COLS = tuple(u * WPK for u in CHUNK_UNITS)

_compiled = None
_custom_op = None


def _register_custom_op():
    """Register the fused AND+any op in the custom-DVE table (idempotent)."""
    global _custom_op
    if _custom_op is not None:
        return _custom_op
    from concourse import dve_ops
    from concourse.dve_spec import Spec, Src0, Src1, Zero, Bin, lower
    from concourse.dve_uop import AluOp, DveOpSpec

    name = "AND_ANY_ANT"
    for o in dve_ops.OPS:
        if o.name == name:
            _custom_op = o
            return o

    def _ref(in0, in1, c0, c1, c2):
        a = in0.view(np.uint32) & in1.view(np.uint32)
        acc = (
            (a.reshape(a.shape[0], -1) != 0)
            .any(axis=-1, keepdims=True)
            .astype(np.float32)
        )
        return a.view(np.float32), acc

    spec = Spec(
        body=Bin(AluOp.BITWISE_AND, Src0, Src1),
        accum=AluOp.LOGICAL_OR,
        accum_init=Zero,
        reference=_ref,
    )
    shas = {}
    for ver in ("v3", "v4"):
        try:
            uops = lower(spec, ver=ver)
            shas[ver] = DveOpSpec(name=name, uops=uops, rd1_en=True).sha(ver)
        except Exception:
            pass
    op = dve_ops.DveOp(name, spec, subdim=False, uops_sha=shas)
    dve_ops.OPS.append(op)
    dve_ops._SUB_OPCODE_FOR_NAME[name] = (
        dve_ops._CUSTOM_DVE_ROW_BASE + len(dve_ops.OPS) - 1
    )
    dve_ops.CUSTOM_DVE_SPECS[name] = spec
    _custom_op = op
    return op


def _build():
    import concourse.bacc as bacc
    import concourse.mybir as mybir
    from concourse import tile

    op = _register_custom_op()

    # Bass.__init__ registers 4 const APs via gpsimd.memset; nothing in this
    # kernel reads them, and their serial Pool-engine memsets delay the
    # kernel-start barrier by ~450ns. Suppress them during construction.
    import concourse.bass as cbass

    orig_memset = cbass.BassGpSimd.memset
    cbass.BassGpSimd.memset = lambda self, ap, constant: None
    try:
        nc = bacc.Bacc(
            "TRN2",
            target_bir_lowering=False,
            debug=False,
            enable_asserts=False,
            num_devices=NCORES,
        )
    finally:
        cbass.BassGpSimd.memset = orig_memset
    TOT = (NB + 1) * WPK
    wx = nc.dram_tensor("wx", [128, TOT], mybir.dt.int32, kind="ExternalInput")
    # kv_writeback-shaped result: [batch=1, dhi=128, dho=1, n_ctx=NB]
    res = nc.dram_tensor(
        "res", [1, 128, 1, NB], mybir.dt.float32, kind="ExternalOutput"
    )

    with tile.TileContext(nc) as tc:
        with (
            tc.tile_pool(name="wpool", bufs=1) as wpool,
            tc.tile_pool(name="small", bufs=1) as small,
        ):
            acc = small.tile([128, NB], mybir.dt.float32)
            idx = small.tile([128, 1], mybir.dt.int32)
            m = small.tile([128, WPK], mybir.dt.float32, tag="m")

            nc.gpsimd.memset(idx[:], 0)
            res_sem = nc.alloc_semaphore("res_dma_sem")

            # Weight + notx stream: HWDGE chunks, alternating SP/Act.
            tiles = []
            c0 = 0
            for ci, cw in enumerate(CHUNK_COLS):
                wt = wpool.tile([128, cw], mybir.dt.int32, tag=f"wt{ci}")
                dma_eng = nc.sync if ci % 2 == 0 else nc.scalar
                dma_eng.dma_start(wt[:], wx[:, c0 : c0 + cw])
                tiles.append((wt, c0, cw))
                c0 += cw

            def words_ap(col, n):
                """fp32-bitcast AP over words [col, col+n) of the stream."""
                for wt, tc0, tcw in tiles:
                    if tc0 <= col and col + n <= tc0 + tcw:
                        return wt[:, col - tc0 : col - tc0 + n].bitcast(
                            mybir.dt.float32
                        )
                raise AssertionError(f"span {col}+{n} crosses a chunk boundary")

            nx_ap = words_ap(0, WPK)  # notx is the stream head
            for gb in range(NB):
                nc.vector._custom_dve(
                    op,
                    out=m[:],
                    in0=words_ap((gb + 1) * WPK, WPK),
                    in1=nx_ap,
                    accum_out=acc[:, gb : gb + 1],
                )
            # Prepared result writeback: the prep only generates descriptors
            # (its RAW dep on `acc` is demoted to a no-sync edge, so the gen
            # runs early on the idle Pool engine); the transfer fires at
            # trigger_dma, which carries the sync deps on the accumulator
            # writes. The tail thus skips HWDGE desc-gen + DGE start delay.
            nc.gpsimd.kv_writeback(
                out_ap=res[:, :, :, :],
                in_ap=acc[:].rearrange("p (a b n) -> p a b n", a=1, b=1),
                ctx_idxs_ap=idx[:],
                prepare_only=True,
                sem=res_sem,
            )
            nc.gpsimd.trigger_dma(count=None)

    # Tile assigns the prep a DMASW0 lane: the end-of-kernel gather waits on
    # DMASW0 >= 16, but pass 2 leaves the descriptor sem slot (on_update[0])
    # at our explicit `sem=`, which nothing then waits on. Retarget the
    # descriptor sem to the Tile-assigned DMASW0 semaphore so the DMA
    # completion ticks the lane the end barrier actually watches.
    swdge_sem_id = None
    prep_inst = None
    prep_blk = None
    for blk in nc.m.functions[0].blocks:
        for inst in blk.instructions:
            if type(inst).__name__ == "InstKVWritebackAnt":
                prep_inst = inst
                prep_blk = blk
            si = inst.sync_info
            if si is not None:
                for w in si.on_wait:
                    if w.ant_name and w.ant_name.startswith("DMASW"):
                        swdge_sem_id = w.id
    assert prep_inst is not None and swdge_sem_id is not None
    upd = prep_inst.sync_info.on_update
    assert upd[0].ant_name == "res_dma_sem"
    upd[0].id = swdge_sem_id
    prep_inst.sync_info.on_update = upd

    # Tile attaches the trigger's data dep (all DVE accumulator writes) to
    # the PREP, which would push the ~1us descriptor-gen onto the
    # post-compute critical path. Desc-gen reads no tensor data, so move
    # the DVE wait from the prep to the trigger: gen runs early on the
    # idle Pool engine and only the transfer launch gates on the DVE ops.
    trig_inst = None
    for inst in prep_blk.instructions:
        if type(inst).__name__ == "InstTriggerDma":
            trig_inst = inst
    prep_si = prep_inst.sync_info
    dve_waits = [
        w for w in prep_si.on_wait if w.ant_name and w.ant_name.startswith("DVE")
    ]
    assert len(dve_waits) == 1 and trig_inst is not None
    prep_si.on_wait = [
        w
        for w in prep_si.on_wait
        if not (w.ant_name and w.ant_name.startswith("DVE"))
    ]
    trig_si = trig_inst.sync_info
    trig_si.on_wait = list(trig_si.on_wait) + dve_waits

    # Strip DMA-completion waits that are already implied by same-engine
    # program order: the second DVE op of a 2-row chunk re-waits the same
    # semaphore threshold the first op already cleared.
    seen: set = set()
    for inst in prep_blk.instructions:
        if type(inst).__name__ != "InstCustomDveAnt":
            continue
        si = inst.sync_info
        if si is None:
            continue
        keep = []
        for w in si.on_wait:
            key = (w.id, w.wait_mode, w.wait_value)
            if key in seen:
                continue
            keep.append(w)
            seen.add(key)
        si.on_wait = keep

    nc.compile()
    return nc


def _pack31(bits):
    """bits [..., D] uint8 -> [..., WPK] int32 words, 31 payload bits per
    word at positions 0..29 and 31 (bit 30 always zero -> never NaN/Inf)."""
    lead = bits.shape[:-1]
    b32 = np.zeros(lead + (WPK, 32), dtype=np.uint8)
    pad = np.zeros(lead + (DPAD,), dtype=np.uint8)
    pad[..., :D] = bits
    pad = pad.reshape(lead + (WPK, PAYLOAD))
    b32[..., :30] = pad[..., :30]
    b32[..., 31] = pad[..., 30]
    words = np.packbits(b32.reshape(lead + (WPK * 32,)), axis=-1, bitorder="little")
    return words.reshape(lead + (WPK * 4,)).view(np.int32)


def _pack_inputs(x, bit_weights):
    x = np.asarray(x).astype(np.uint8)
    bw = np.ascontiguousarray(np.asarray(bit_weights).astype(np.uint8))
    notx = (1 - x).astype(np.uint8)
    nxp = _pack31(notx)  # [WPK]
    wp = _pack31(bw)  # [L, WPK]
    in_maps = []
    for i in range(NCORES):
        shard = wp[i * LSH : (i + 1) * LSH].reshape(128, NB, WPK)
        wx = np.empty((128, NB + 1, WPK), dtype=np.int32)
        wx[:, 0, :] = nxp
        wx[:, 1:, :] = shard
        in_maps.append({"wx": wx.reshape(128, (NB + 1) * WPK)})
    return in_maps


def _gather(results):
    outs = []
    for i in range(NCORES):
        # [1, 128, 1, NB] fp32 violation flags; res[0, p, 0, b] covers
        # neuron 8p + b; flag == 0.0 means no violated requirement -> True
        res = results[i]["res"].reshape(128, NB)
        outs.append(res.reshape(-1) == 0.0)
    return np.concatenate(outs).astype(np.bool_)


def _get_compiled():
    global _compiled
    if _compiled is None:
        _compiled = _build()
    return _compiled


def kernel(x, bit_weights):
    from concourse import bass_utils

    nc = _get_compiled()
    in_maps = _pack_inputs(x, bit_weights)
    r = bass_utils.run_bass_kernel_spmd(nc, in_maps, core_ids=list(range(NCORES)))
    return _gather(r.results)


# revision 33
# speedup vs baseline: 1.1755x; 1.0007x over previous
"""HardAndLayer on 8 Trainium2 NeuronCores.

out[l] = AND_d (x[d] OR NOT w[l,d])  ==  no d with (w[l,d] AND NOT x[d])

Strategy (per sharding hint): shard bit_weights row-wise (neuron dim) across
8 cores, x replicated, no collectives.

Wire format: bools are bit-packed 31 per int32 word with bit 30 forced
zero, so no word can form an fp32 NaN/Inf pattern (the DVE fp32 stream
path canonicalizes NaN operands — HW-verified failure with full 32-bit
packing). DRAM tensors are declared int32; on device the SBUF APs are
bitcast to fp32 for the custom DVE op, whose datapath is bitwise: per row
    acc[p] = fold_logical_or_j (w_packed[p, j] BITWISE_AND notx_packed[j])
(identity fp32 converter, BITWISE_AND preserves raw bits, LOGICAL_OR folds
on bit-pattern truthiness; -0.0/NaN/denormal patterns all count as nonzero).
out[l] = (acc == 0), applied on the host to the DMA'd per-neuron flags.

Pipeline: the weight shard streams in 4 HWDGE DMA chunks (each chunk->DVE
edge re-pays the ~900ns DMA-semaphore latency, so sizes are chosen to
keep the per-chunk anchors A_k = arrival_k + 900 + dve_work_after_k flat;
more chunks would serialize on the single 625ns/instr HWDGE). The result
write is a prepared SWDGE kv_writeback fired by trigger_dma right after
the last DVE accumulator write: its descriptors are generated early, off
the critical path, so the tail skips HWDGE descriptor-gen and the DGE
start delay entirely.

Layout: partition p of a core holds its 8 consecutive neuron rows
(8 KB contiguous per partition); res[p, b] = out[8p + b].
"""

import numpy as np

L = 8192
D = 8192
NCORES = 8
LSH = L // NCORES  # 1024 neuron rows per core
PAYLOAD = 31  # bits per packed word; bit 30 held zero -> never NaN/Inf.
# The DVE fp32 stream path canonicalizes NaN-pattern operands (HW-verified:
# 32-bit packing fails exactly on words with the exponent field all-ones),
# so the wire format must never form one. Denormals and -0.0 pass bit-exact.
WPK = -(-D // PAYLOAD)  # 265 packed words per neuron row
DPAD = WPK * PAYLOAD
_BITPOS = list(range(30)) + [31]  # payload bit positions (skip bit 30)
NB = LSH // 128  # 8 neuron rows per partition
# Per-partition DRAM layout: [notx | row0 | ... | row7], 9*WPK words.
# Chunk sizes in row units (1 unit = WPK words = ~1 KiB/partition):
# (notx,r0,r1 | r2,r3 | r4,r5 | r6,r7) -- flat anchor schedule; 4 chunks
# keep the serial HWDGE descriptor-gen (625ns/instr) off the DMA stream.
CHUNK_UNITS = (3, 2, 2, 2)
assert sum(CHUNK_UNITS) == NB + 1
CHUNK_COLS = tuple(u * WPK for u in CHUNK_UNITS)

_compiled = None
_custom_op = None


def _register_custom_op():
    """Register the fused AND+any op in the custom-DVE table (idempotent)."""
    global _custom_op
    if _custom_op is not None:
        return _custom_op
    from concourse import dve_ops
    from concourse.dve_spec import Spec, Src0, Src1, Zero, Bin, lower
    from concourse.dve_uop import AluOp, DveOpSpec

    name = "AND_ANY_ANT"
    for o in dve_ops.OPS:
        if o.name == name:
            _custom_op = o
            return o

    def _ref(in0, in1, c0, c1, c2):
        a = in0.view(np.uint32) & in1.view(np.uint32)
        acc = (
            (a.reshape(a.shape[0], -1) != 0)
            .any(axis=-1, keepdims=True)
            .astype(np.float32)
        )
        return a.view(np.float32), acc

    spec = Spec(
        body=Bin(AluOp.BITWISE_AND, Src0, Src1),
        accum=AluOp.LOGICAL_OR,
        accum_init=Zero,
        reference=_ref,
    )
    shas = {}
    for ver in ("v3", "v4"):
        try:
            uops = lower(spec, ver=ver)
            shas[ver] = DveOpSpec(name=name, uops=uops, rd1_en=True).sha(ver)
        except Exception:
            pass
    op = dve_ops.DveOp(name, spec, subdim=False, uops_sha=shas)
    dve_ops.OPS.append(op)
    dve_ops._SUB_OPCODE_FOR_NAME[name] = (
        dve_ops._CUSTOM_DVE_ROW_BASE + len(dve_ops.OPS) - 1
    )
    dve_ops.CUSTOM_DVE_SPECS[name] = spec
    _custom_op = op
    return op


def _build():
    import concourse.bacc as bacc
    import concourse.mybir as mybir
    from concourse import tile

    op = _register_custom_op()

    # Bass.__init__ registers 4 const APs via gpsimd.memset; nothing in this
    # kernel reads them, and their serial Pool-engine memsets delay the
    # kernel-start barrier by ~450ns. Suppress them during construction.
    import concourse.bass as cbass

    orig_memset = cbass.BassGpSimd.memset
    cbass.BassGpSimd.memset = lambda self, ap, constant: None
    try:
        nc = bacc.Bacc(
            "TRN2",
            target_bir_lowering=False,
            debug=False,
            enable_asserts=False,
            num_devices=NCORES,
        )
    finally:
        cbass.BassGpSimd.memset = orig_memset
    TOT = (NB + 1) * WPK
    wx = nc.dram_tensor("wx", [128, TOT], mybir.dt.int32, kind="ExternalInput")
    # kv_writeback-shaped result: [batch=1, dhi=128, dho=1, n_ctx=NB]
    res = nc.dram_tensor(
        "res", [1, 128, 1, NB], mybir.dt.float32, kind="ExternalOutput"
    )

    with tile.TileContext(nc) as tc:
        with (
            tc.tile_pool(name="wpool", bufs=1) as wpool,
            tc.tile_pool(name="small", bufs=1) as small,
        ):
            acc = small.tile([128, NB], mybir.dt.float32)
            idx = small.tile([128, 1], mybir.dt.int32)
            m = small.tile([128, WPK], mybir.dt.float32, tag="m")

            nc.gpsimd.memset(idx[:], 0)
            res_sem = nc.alloc_semaphore("res_dma_sem")

            # Weight + notx stream: HWDGE chunks, alternating SP/Act.
            tiles = []
            c0 = 0
            for ci, cw in enumerate(CHUNK_COLS):
                wt = wpool.tile([128, cw], mybir.dt.int32, tag=f"wt{ci}")
                dma_eng = nc.sync if ci % 2 == 0 else nc.scalar
                dma_eng.dma_start(wt[:], wx[:, c0 : c0 + cw])
                tiles.append((wt, c0, cw))
                c0 += cw

            def words_ap(col, n):
                """fp32-bitcast AP over words [col, col+n) of the stream."""
                for wt, tc0, tcw in tiles:
                    if tc0 <= col and col + n <= tc0 + tcw:
                        return wt[:, col - tc0 : col - tc0 + n].bitcast(
                            mybir.dt.float32
                        )
                raise AssertionError(f"span {col}+{n} crosses a chunk boundary")

            nx_ap = words_ap(0, WPK)  # notx is the stream head
            for gb in range(NB):
                nc.vector._custom_dve(
                    op,
                    out=m[:],
                    in0=words_ap((gb + 1) * WPK, WPK),
                    in1=nx_ap,
                    accum_out=acc[:, gb : gb + 1],
                )
            # Prepared result writeback: the prep only generates descriptors
            # (its RAW dep on `acc` is demoted to a no-sync edge, so the gen
            # runs early on the idle Pool engine); the transfer fires at
            # trigger_dma, which carries the sync deps on the accumulator
            # writes. The tail thus skips HWDGE desc-gen + DGE start delay.
            nc.gpsimd.kv_writeback(
                out_ap=res[:, :, :, :],
                in_ap=acc[:].rearrange("p (a b n) -> p a b n", a=1, b=1),
                ctx_idxs_ap=idx[:],
                prepare_only=True,
                sem=res_sem,
            )
            nc.gpsimd.trigger_dma(count=None)

    # Tile assigns the prep a DMASW0 lane: the end-of-kernel gather waits on
    # DMASW0 >= 16, but pass 2 leaves the descriptor sem slot (on_update[0])
    # at our explicit `sem=`, which nothing then waits on. Retarget the
    # descriptor sem to the Tile-assigned DMASW0 semaphore so the DMA
    # completion ticks the lane the end barrier actually watches.
    swdge_sem_id = None
    prep_inst = None
    prep_blk = None
    for blk in nc.m.functions[0].blocks:
        for inst in blk.instructions:
            if type(inst).__name__ == "InstKVWritebackAnt":
                prep_inst = inst
                prep_blk = blk
            si = inst.sync_info
            if si is not None:
                for w in si.on_wait:
                    if w.ant_name and w.ant_name.startswith("DMASW"):
                        swdge_sem_id = w.id
    assert prep_inst is not None and swdge_sem_id is not None
    upd = prep_inst.sync_info.on_update
    assert upd[0].ant_name == "res_dma_sem"
    upd[0].id = swdge_sem_id
    prep_inst.sync_info.on_update = upd

    # Tile attaches the trigger's data dep (all DVE accumulator writes) to
    # the PREP, which would push the ~1us descriptor-gen onto the
    # post-compute critical path. Desc-gen reads no tensor data, so move
    # the DVE wait from the prep to the trigger: gen runs early on the
    # idle Pool engine and only the transfer launch gates on the DVE ops.
    trig_inst = None
    for inst in prep_blk.instructions:
        if type(inst).__name__ == "InstTriggerDma":
            trig_inst = inst
    prep_si = prep_inst.sync_info
    dve_waits = [
        w for w in prep_si.on_wait if w.ant_name and w.ant_name.startswith("DVE")
    ]
    assert len(dve_waits) == 1 and trig_inst is not None
    prep_si.on_wait = [
        w
        for w in prep_si.on_wait
        if not (w.ant_name and w.ant_name.startswith("DVE"))
    ]
    trig_si = trig_inst.sync_info
    trig_si.on_wait = list(trig_si.on_wait) + dve_waits

    # Strip DMA-completion waits that are already implied by same-engine
    # program order: the second DVE op of a 2-row chunk re-waits the same
    # semaphore threshold the first op already cleared.
    seen: set = set()
    for inst in prep_blk.instructions:
        if type(inst).__name__ != "InstCustomDveAnt":
            continue
        si = inst.sync_info
        if si is None:
            continue
        keep = []
        for w in si.on_wait:
            key = (w.id, w.wait_mode, w.wait_value)
            if key in seen:
                continue
            keep.append(w)
            seen.add(key)
        si.on_wait = keep

    nc.compile()
    return nc


def _pack31(bits):
    """bits [..., D] uint8 -> [..., WPK] int32 words, 31 payload bits per
    word at positions 0..29 and 31 (bit 30 always zero -> never NaN/Inf)."""
    lead = bits.shape[:-1]
    b32 = np.zeros(lead + (WPK, 32), dtype=np.uint8)
    pad = np.zeros(lead + (DPAD,), dtype=np.uint8)
    pad[..., :D] = bits
    pad = pad.reshape(lead + (WPK, PAYLOAD))
    b32[..., :30] = pad[..., :30]
    b32[..., 31] = pad[..., 30]
    words = np.packbits(b32.reshape(lead + (WPK * 32,)), axis=-1, bitorder="little")
    return words.reshape(lead + (WPK * 4,)).view(np.int32)


def _pack_inputs(x, bit_weights):
    x = np.asarray(x).astype(np.uint8)
    bw = np.ascontiguousarray(np.asarray(bit_weights).astype(np.uint8))
    notx = (1 - x).astype(np.uint8)
    nxp = _pack31(notx)  # [WPK]
    wp = _pack31(bw)  # [L, WPK]
    in_maps = []
    for i in range(NCORES):
        shard = wp[i * LSH : (i + 1) * LSH].reshape(128, NB, WPK)
        wx = np.empty((128, NB + 1, WPK), dtype=np.int32)
        wx[:, 0, :] = nxp
        wx[:, 1:, :] = shard
        in_maps.append({"wx": wx.reshape(128, (NB + 1) * WPK)})
    return in_maps


def _gather(results):
    outs = []
    for i in range(NCORES):
        # [1, 128, 1, NB] fp32 violation flags; res[0, p, 0, b] covers
        # neuron 8p + b; flag == 0.0 means no violated requirement -> True
        res = results[i]["res"].reshape(128, NB)
        outs.append(res.reshape(-1) == 0.0)
    return np.concatenate(outs).astype(np.bool_)


def _get_compiled():
    global _compiled
    if _compiled is None:
        _compiled = _build()
    return _compiled


def kernel(x, bit_weights):
    from concourse import bass_utils

    nc = _get_compiled()
    in_maps = _pack_inputs(x, bit_weights)
    r = bass_utils.run_bass_kernel_spmd(nc, in_maps, core_ids=list(range(NCORES)))
    return _gather(r.results)


# revision 43
# speedup vs baseline: 1.1772x; 1.0014x over previous
"""HardAndLayer on 8 Trainium2 NeuronCores.

out[l] = AND_d (x[d] OR NOT w[l,d])  ==  no d with (w[l,d] AND NOT x[d])

Strategy (per sharding hint): shard bit_weights row-wise (neuron dim) across
8 cores, x replicated, no collectives.

Wire format: bools are bit-packed 31 per int32 word with bit 30 forced
zero, so no word can form an fp32 NaN/Inf pattern (the DVE fp32 stream
path canonicalizes NaN operands — HW-verified failure with full 32-bit
packing). DRAM tensors are declared int32; on device the SBUF APs are
bitcast to fp32 for the custom DVE op, whose datapath is bitwise: per row
    acc[p] = fold_logical_or_j (w_packed[p, j] BITWISE_AND notx_packed[j])
(identity fp32 converter, BITWISE_AND preserves raw bits, LOGICAL_OR folds
on bit-pattern truthiness; -0.0/NaN/denormal patterns all count as nonzero).
out[l] = (acc == 0), applied on the host to the DMA'd per-neuron flags.

Pipeline: the weight shard streams in 4 HWDGE DMA chunks (each chunk->DVE
edge re-pays the ~900ns DMA-semaphore latency, so sizes are chosen to
keep the per-chunk anchors A_k = arrival_k + 900 + dve_work_after_k flat;
more chunks would serialize on the single 625ns/instr HWDGE). The result
write is a prepared SWDGE kv_writeback fired by trigger_dma right after
the last DVE accumulator write: its descriptors are generated early, off
the critical path, so the tail skips HWDGE descriptor-gen and the DGE
start delay entirely.

Layout: partition p of a core holds its 8 consecutive neuron rows
(8 KB contiguous per partition); res[p, b] = out[8p + b].
"""

import numpy as np

L = 8192
D = 8192
NCORES = 8
LSH = L // NCORES  # 1024 neuron rows per core
PAYLOAD = 31  # bits per packed word; bit 30 held zero -> never NaN/Inf.
# The DVE fp32 stream path canonicalizes NaN-pattern operands (HW-verified:
# 32-bit packing fails exactly on words with the exponent field all-ones),
# so the wire format must never form one. Denormals and -0.0 pass bit-exact.
WPK = -(-D // PAYLOAD)  # 265 packed words per neuron row
DPAD = WPK * PAYLOAD
NB = LSH // 128  # 8 neuron rows per partition
# Per-partition DRAM layout: [notx | row0 | ... | row7], 9*WPK words.
# Chunk sizes in row units (1 unit = WPK words = ~1 KiB/partition):
# (notx,r0,r1 | r2,r3 | r4,r5 | r6,r7) -- flat anchor schedule; 4 chunks
# keep the serial HWDGE descriptor-gen (625ns/instr) off the DMA stream.
CHUNK_UNITS = (3, 2, 2, 1, 1)
assert sum(CHUNK_UNITS) == NB + 1
CHUNK_COLS = tuple(u * WPK for u in CHUNK_UNITS)

_compiled = None
_custom_op = None


def _register_custom_op():
    """Register the fused AND+any op in the custom-DVE table (idempotent)."""
    global _custom_op
    if _custom_op is not None:
        return _custom_op
    from concourse import dve_ops
    from concourse.dve_spec import Spec, Src0, Src1, Zero, Bin, lower
    from concourse.dve_uop import AluOp, DveOpSpec

    name = "AND_ANY_ANT"
    for o in dve_ops.OPS:
        if o.name == name:
            _custom_op = o
            return o

    def _ref(in0, in1, c0, c1, c2):
        a = in0.view(np.uint32) & in1.view(np.uint32)
        acc = (
            (a.reshape(a.shape[0], -1) != 0)
            .any(axis=-1, keepdims=True)
            .astype(np.float32)
        )
        return a.view(np.float32), acc

    spec = Spec(
        body=Bin(AluOp.BITWISE_AND, Src0, Src1),
        accum=AluOp.LOGICAL_OR,
        accum_init=Zero,
        reference=_ref,
    )
    shas = {}
    for ver in ("v3", "v4"):
        try:
            uops = lower(spec, ver=ver)
            shas[ver] = DveOpSpec(name=name, uops=uops, rd1_en=True).sha(ver)
        except Exception:
            pass
    op = dve_ops.DveOp(name, spec, subdim=False, uops_sha=shas)
    dve_ops.OPS.append(op)
    dve_ops._SUB_OPCODE_FOR_NAME[name] = (
        dve_ops._CUSTOM_DVE_ROW_BASE + len(dve_ops.OPS) - 1
    )
    dve_ops.CUSTOM_DVE_SPECS[name] = spec
    _custom_op = op
    return op


def _build():
    import concourse.bacc as bacc
    import concourse.mybir as mybir
    from concourse import tile

    op = _register_custom_op()

    # Bass.__init__ registers 4 const APs via gpsimd.memset; nothing in this
    # kernel reads them, and their serial Pool-engine memsets delay the
    # kernel-start barrier by ~450ns. Suppress them during construction.
    import concourse.bass as cbass

    orig_memset = cbass.BassGpSimd.memset
    cbass.BassGpSimd.memset = lambda self, ap, constant: None
    try:
        nc = bacc.Bacc(
            "TRN2",
            target_bir_lowering=False,
            debug=False,
            enable_asserts=False,
            num_devices=NCORES,
        )
    finally:
        cbass.BassGpSimd.memset = orig_memset
    TOT = (NB + 1) * WPK
    wx = nc.dram_tensor("wx", [128, TOT], mybir.dt.int32, kind="ExternalInput")
    # kv_writeback-shaped result: [batch=1, dhi=128, dho=1, n_ctx=NB]
    res = nc.dram_tensor(
        "res", [1, 128, 1, NB], mybir.dt.float32, kind="ExternalOutput"
    )

    with tile.TileContext(nc) as tc:
        with (
            tc.tile_pool(name="wpool", bufs=1) as wpool,
            tc.tile_pool(name="small", bufs=1) as small,
        ):
            acc = small.tile([128, NB], mybir.dt.float32)
            idx = small.tile([128, 1], mybir.dt.int32)
            m = small.tile([128, WPK], mybir.dt.float32, tag="m")
            nc.gpsimd.memset(idx[:], 0)
            res_sem = nc.alloc_semaphore("res_dma_sem")

            # Weight + notx stream: HWDGE chunks, alternating SP/Act.
            tiles = []
            c0 = 0
            for ci, cw in enumerate(CHUNK_COLS):
                wt = wpool.tile([128, cw], mybir.dt.int32, tag=f"wt{ci}")
                dma_eng = nc.sync if ci % 2 == 0 else nc.scalar
                dma_eng.dma_start(wt[:], wx[:, c0 : c0 + cw])
                tiles.append((wt, c0, cw))
                c0 += cw

            def words_ap(col, n):
                """fp32-bitcast AP over words [col, col+n) of the stream."""
                for wt, tc0, tcw in tiles:
                    if tc0 <= col and col + n <= tc0 + tcw:
                        return wt[:, col - tc0 : col - tc0 + n].bitcast(
                            mybir.dt.float32
                        )
                raise AssertionError(f"span {col}+{n} crosses a chunk boundary")

            nx_ap = words_ap(0, WPK)  # notx is the stream head
            for gb in range(NB):
                nc.vector._custom_dve(
                    op,
                    out=m[:],
                    in0=words_ap((gb + 1) * WPK, WPK),
                    in1=nx_ap,
                    accum_out=acc[:, gb : gb + 1],
                )
            # Prepared result writeback: the prep only generates descriptors
            # (its RAW dep on `acc` is demoted to a no-sync edge, so the gen
            # runs early on the idle Pool engine); the transfer fires at
            # trigger_dma, which carries the sync deps on the accumulator
            # writes. The tail thus skips HWDGE desc-gen + DGE start delay.
            nc.gpsimd.kv_writeback(
                out_ap=res[:, :, :, :],
                in_ap=acc[:].rearrange("p (a b n) -> p a b n", a=1, b=1),
                ctx_idxs_ap=idx[:],
                prepare_only=True,
                sem=res_sem,
            )
            nc.gpsimd.trigger_dma(count=None)

    # Tile assigns each SWDGE prep a DMASW lane: consumers and the
    # end-of-kernel gather wait on DMASW<k> >= 16, but pass 2 leaves the
    # descriptor sem slot (on_update[0]) at our explicit `sem=`, which
    # nothing then waits on. Retarget each prep's descriptor sem to its
    # Tile-assigned lane semaphore (lanes are assigned to Pool DMA
    # instructions in program order) so the DMA completions tick the
    # lanes the waits actually watch.
    dmasw_ids = {}
    preps = []
    prep_blk = None
    for blk in nc.m.functions[0].blocks:
        for inst in blk.instructions:
            tn = type(inst).__name__
            if (
                tn in ("InstKVWritebackAnt", "InstDMAGatherAnt")
                and getattr(inst, "gen_mode", 0) == 1
            ):
                preps.append(inst)
                prep_blk = blk
            si = inst.sync_info
            if si is not None:
                for w in si.on_wait:
                    if w.ant_name and w.ant_name.startswith("DMASW"):
                        dmasw_ids[w.ant_name.split("_")[0]] = w.id
    assert len(preps) == 1 and dmasw_ids
    nlanes = len(dmasw_ids)
    for k, prep in enumerate(preps):
        upd = prep.sync_info.on_update
        assert upd[0].ant_name in ("nx_dma_sem", "res_dma_sem")
        upd[0].id = dmasw_ids[f"DMASW{k % nlanes}"]
        prep.sync_info.on_update = upd

    # Tile attaches the result-trigger's data dep (all DVE accumulator
    # writes) to the kv_writeback PREP, which would push the ~1us
    # descriptor-gen onto the post-compute critical path. Desc-gen reads
    # no tensor data, so move the DVE wait from the prep to the final
    # trigger: gen runs early on the idle Pool engine and only the
    # transfer launch gates on the DVE ops.
    kv_prep = preps[0]
    assert type(kv_prep).__name__ == "InstKVWritebackAnt"
    trig_inst = None
    for inst in prep_blk.instructions:
        if type(inst).__name__ == "InstTriggerDma":
            trig_inst = inst  # keep last
    prep_si = kv_prep.sync_info
    dve_waits = [
        w for w in prep_si.on_wait if w.ant_name and w.ant_name.startswith("DVE")
    ]
    assert len(dve_waits) == 1 and trig_inst is not None
    prep_si.on_wait = [
        w
        for w in prep_si.on_wait
        if not (w.ant_name and w.ant_name.startswith("DVE"))
    ]
    trig_si = trig_inst.sync_info
    trig_si.on_wait = list(trig_si.on_wait) + dve_waits

    # Strip DMA-completion waits that are already implied by same-engine
    # program order: the second DVE op of a 2-row chunk re-waits the same
    # semaphore threshold the first op already cleared.
    seen: set = set()
    for inst in prep_blk.instructions:
        if type(inst).__name__ != "InstCustomDveAnt":
            continue
        si = inst.sync_info
        if si is None:
            continue
        keep = []
        for w in si.on_wait:
            key = (w.id, w.wait_mode, w.wait_value)
            if key in seen:
                continue
            keep.append(w)
            seen.add(key)
        si.on_wait = keep

    nc.compile()
    return nc


def _pack31(bits):
    """bits [..., D] uint8 -> [..., WPK] int32 words, 31 payload bits per
    word at positions 0..29 and 31 (bit 30 always zero -> never NaN/Inf)."""
    lead = bits.shape[:-1]
    b32 = np.zeros(lead + (WPK, 32), dtype=np.uint8)
    pad = np.zeros(lead + (DPAD,), dtype=np.uint8)
    pad[..., :D] = bits
    pad = pad.reshape(lead + (WPK, PAYLOAD))
    b32[..., :30] = pad[..., :30]
    b32[..., 31] = pad[..., 30]
    words = np.packbits(b32.reshape(lead + (WPK * 32,)), axis=-1, bitorder="little")
    return words.reshape(lead + (WPK * 4,)).view(np.int32)


def _pack_inputs(x, bit_weights):
    x = np.asarray(x).astype(np.uint8)
    bw = np.ascontiguousarray(np.asarray(bit_weights).astype(np.uint8))
    notx = (1 - x).astype(np.uint8)
    nxp = _pack31(notx)  # [WPK]
    wp = _pack31(bw)  # [L, WPK]
    in_maps = []
    for i in range(NCORES):
        shard = wp[i * LSH : (i + 1) * LSH].reshape(128, NB, WPK)
        wx = np.empty((128, NB + 1, WPK), dtype=np.int32)
        wx[:, 0, :] = nxp
        wx[:, 1:, :] = shard
        in_maps.append({"wx": wx.reshape(128, (NB + 1) * WPK)})
    return in_maps


def _gather(results):
    outs = []
    for i in range(NCORES):
        # [1, 128, 1, NB] fp32 violation flags; res[0, p, 0, b] covers
        # neuron 8p + b; flag == 0.0 means no violated requirement -> True
        res = results[i]["res"].reshape(128, NB)
        outs.append(res.reshape(-1) == 0.0)
    return np.concatenate(outs).astype(np.bool_)


def _get_compiled():
    global _compiled
    if _compiled is None:
        _compiled = _build()
    return _compiled


def kernel(x, bit_weights):
    from concourse import bass_utils

    nc = _get_compiled()
    in_maps = _pack_inputs(x, bit_weights)
    r = bass_utils.run_bass_kernel_spmd(nc, in_maps, core_ids=list(range(NCORES)))
    return _gather(r.results)


# revision 44
# speedup vs baseline: 1.2137x; 1.0310x over previous
"""HardAndLayer on 8 Trainium2 NeuronCores.

out[l] = AND_d (x[d] OR NOT w[l,d])  ==  no d with (w[l,d] AND NOT x[d])

Strategy (per sharding hint): shard bit_weights row-wise (neuron dim) across
8 cores, x replicated, no collectives.

Wire format: bools are bit-packed 31 per int32 word with bit 30 forced
zero, so no word can form an fp32 NaN/Inf pattern (the DVE fp32 stream
path canonicalizes NaN operands — HW-verified failure with full 32-bit
packing). DRAM tensors are declared int32; on device the SBUF APs are
bitcast to fp32 for the custom DVE op, whose datapath is bitwise: per row
    acc[p] = fold_logical_or_j (w_packed[p, j] BITWISE_AND notx_packed[j])
(identity fp32 converter, BITWISE_AND preserves raw bits, LOGICAL_OR folds
on bit-pattern truthiness; -0.0/NaN/denormal patterns all count as nonzero).
out[l] = (acc == 0), applied on the host to the DMA'd per-neuron flags.

Pipeline: the weight shard streams in 4 HWDGE DMA chunks (each chunk->DVE
edge re-pays the ~900ns DMA-semaphore latency, so sizes are chosen to
keep the per-chunk anchors A_k = arrival_k + 900 + dve_work_after_k flat;
more chunks would serialize on the single 625ns/instr HWDGE). The result
write is a prepared SWDGE kv_writeback fired by trigger_dma right after
the last DVE accumulator write: its descriptors are generated early, off
the critical path, so the tail skips HWDGE descriptor-gen and the DGE
start delay entirely.

Layout: partition p of a core holds its 8 consecutive neuron rows
(8 KB contiguous per partition); res[p, b] = out[8p + b].
"""

import numpy as np

L = 8192
D = 8192
NCORES = 8
LSH = L // NCORES  # 1024 neuron rows per core
PAYLOAD = 31  # bits per packed word; bit 30 held zero -> never NaN/Inf.
# The DVE fp32 stream path canonicalizes NaN-pattern operands (HW-verified:
# 32-bit packing fails exactly on words with the exponent field all-ones),
# so the wire format must never form one. Denormals and -0.0 pass bit-exact.
WPK = -(-D // PAYLOAD)  # 265 packed words per neuron row
DPAD = WPK * PAYLOAD
NB = LSH // 128  # 8 neuron rows per partition
# Per-partition DRAM layout: [notx | row0 | ... | row7], 9*WPK words.
# Chunk sizes in row units (1 unit = WPK words = ~1 KiB/partition):
# (notx,r0,r1 | r2,r3 | r4,r5 | r6,r7) -- flat anchor schedule; 4 chunks
# keep the serial HWDGE descriptor-gen (625ns/instr) off the DMA stream.
CHUNK_UNITS = (3, 2, 2, 1, 1)
assert sum(CHUNK_UNITS) == NB + 1
CHUNK_COLS = tuple(u * WPK for u in CHUNK_UNITS)

_compiled = None
_custom_op = None


def _register_custom_op():
    """Register the fused AND+any op in the custom-DVE table (idempotent)."""
    global _custom_op
    if _custom_op is not None:
        return _custom_op
    from concourse import dve_ops
    from concourse.dve_spec import Spec, Src0, Src1, Zero, Bin, lower
    from concourse.dve_uop import AluOp, DveOpSpec

    name = "AND_ANY_ANT"
    for o in dve_ops.OPS:
        if o.name == name:
            _custom_op = o
            return o

    def _ref(in0, in1, c0, c1, c2):
        a = in0.view(np.uint32) & in1.view(np.uint32)
        acc = (
            (a.reshape(a.shape[0], -1) != 0)
            .any(axis=-1, keepdims=True)
            .astype(np.float32)
        )
        return a.view(np.float32), acc

    spec = Spec(
        body=Bin(AluOp.BITWISE_AND, Src0, Src1),
        accum=AluOp.LOGICAL_OR,
        accum_init=Zero,
        reference=_ref,
    )
    shas = {}
    for ver in ("v3", "v4"):
        try:
            uops = lower(spec, ver=ver)
            shas[ver] = DveOpSpec(name=name, uops=uops, rd1_en=True).sha(ver)
        except Exception:
            pass
    op = dve_ops.DveOp(name, spec, subdim=False, uops_sha=shas)
    dve_ops.OPS.append(op)
    dve_ops._SUB_OPCODE_FOR_NAME[name] = (
        dve_ops._CUSTOM_DVE_ROW_BASE + len(dve_ops.OPS) - 1
    )
    dve_ops.CUSTOM_DVE_SPECS[name] = spec
    _custom_op = op
    return op


def _build():
    import concourse.bacc as bacc
    import concourse.mybir as mybir
    from concourse import tile

    op = _register_custom_op()

    # Bass.__init__ registers 4 const APs via gpsimd.memset and emits a
    # kernel-start all-engine barrier. Nothing in this kernel reads the
    # const APs, and every cross-engine edge here is semaphore-gated (DMA
    # completion sems / engine ticks), so neither is needed; together they
    # delay the first DMA by ~700ns. Suppress both during construction.
    import concourse.bass as cbass

    orig_memset = cbass.BassGpSimd.memset
    orig_barrier = cbass.Bass.all_engine_barrier
    cbass.BassGpSimd.memset = lambda self, ap, constant: None
    cbass.Bass.all_engine_barrier = lambda self, *a, **k: None
    try:
        nc = bacc.Bacc(
            "TRN2",
            target_bir_lowering=False,
            debug=False,
            enable_asserts=False,
            num_devices=NCORES,
        )
    finally:
        cbass.BassGpSimd.memset = orig_memset
        cbass.Bass.all_engine_barrier = orig_barrier
    TOT = (NB + 1) * WPK
    wx = nc.dram_tensor("wx", [128, TOT], mybir.dt.int32, kind="ExternalInput")
    # kv_writeback-shaped result: [batch=1, dhi=128, dho=1, n_ctx=NB]
    res = nc.dram_tensor(
        "res", [1, 128, 1, NB], mybir.dt.float32, kind="ExternalOutput"
    )

    with tile.TileContext(nc) as tc:
        with (
            tc.tile_pool(name="wpool", bufs=1) as wpool,
            tc.tile_pool(name="small", bufs=1) as small,
        ):
            acc = small.tile([128, NB], mybir.dt.float32)
            idx = small.tile([128, 1], mybir.dt.int32)
            m = small.tile([128, WPK], mybir.dt.float32, tag="m")
            nc.gpsimd.memset(idx[:], 0)
            res_sem = nc.alloc_semaphore("res_dma_sem")

            # Weight + notx stream: HWDGE chunks, alternating SP/Act.
            tiles = []
            c0 = 0
            for ci, cw in enumerate(CHUNK_COLS):
                wt = wpool.tile([128, cw], mybir.dt.int32, tag=f"wt{ci}")
                dma_eng = nc.sync if ci % 2 == 0 else nc.scalar
                dma_eng.dma_start(wt[:], wx[:, c0 : c0 + cw])
                tiles.append((wt, c0, cw))
                c0 += cw

            def words_ap(col, n):
                """fp32-bitcast AP over words [col, col+n) of the stream."""
                for wt, tc0, tcw in tiles:
                    if tc0 <= col and col + n <= tc0 + tcw:
                        return wt[:, col - tc0 : col - tc0 + n].bitcast(
                            mybir.dt.float32
                        )
                raise AssertionError(f"span {col}+{n} crosses a chunk boundary")

            nx_ap = words_ap(0, WPK)  # notx is the stream head
            for gb in range(NB):
                nc.vector._custom_dve(
                    op,
                    out=m[:],
                    in0=words_ap((gb + 1) * WPK, WPK),
                    in1=nx_ap,
                    accum_out=acc[:, gb : gb + 1],
                )
            # Prepared result writeback: the prep only generates descriptors
            # (its RAW dep on `acc` is demoted to a no-sync edge, so the gen
            # runs early on the idle Pool engine); the transfer fires at
            # trigger_dma, which carries the sync deps on the accumulator
            # writes. The tail thus skips HWDGE desc-gen + DGE start delay.
            nc.gpsimd.kv_writeback(
                out_ap=res[:, :, :, :],
                in_ap=acc[:].rearrange("p (a b n) -> p a b n", a=1, b=1),
                ctx_idxs_ap=idx[:],
                prepare_only=True,
                sem=res_sem,
            )
            nc.gpsimd.trigger_dma(count=None)

    # Tile assigns each SWDGE prep a DMASW lane: consumers and the
    # end-of-kernel gather wait on DMASW<k> >= 16, but pass 2 leaves the
    # descriptor sem slot (on_update[0]) at our explicit `sem=`, which
    # nothing then waits on. Retarget each prep's descriptor sem to its
    # Tile-assigned lane semaphore (lanes are assigned to Pool DMA
    # instructions in program order) so the DMA completions tick the
    # lanes the waits actually watch.
    dmasw_ids = {}
    preps = []
    prep_blk = None
    for blk in nc.m.functions[0].blocks:
        for inst in blk.instructions:
            tn = type(inst).__name__
            if (
                tn in ("InstKVWritebackAnt", "InstDMAGatherAnt")
                and getattr(inst, "gen_mode", 0) == 1
            ):
                preps.append(inst)
                prep_blk = blk
            si = inst.sync_info
            if si is not None:
                for w in si.on_wait:
                    if w.ant_name and w.ant_name.startswith("DMASW"):
                        dmasw_ids[w.ant_name.split("_")[0]] = w.id
    assert len(preps) == 1 and dmasw_ids
    nlanes = len(dmasw_ids)
    for k, prep in enumerate(preps):
        upd = prep.sync_info.on_update
        assert upd[0].ant_name in ("nx_dma_sem", "res_dma_sem")
        upd[0].id = dmasw_ids[f"DMASW{k % nlanes}"]
        prep.sync_info.on_update = upd

    # Tile attaches the result-trigger's data dep (all DVE accumulator
    # writes) to the kv_writeback PREP, which would push the ~1us
    # descriptor-gen onto the post-compute critical path. Desc-gen reads
    # no tensor data, so move the DVE wait from the prep to the final
    # trigger: gen runs early on the idle Pool engine and only the
    # transfer launch gates on the DVE ops.
    kv_prep = preps[0]
    assert type(kv_prep).__name__ == "InstKVWritebackAnt"
    trig_inst = None
    for inst in prep_blk.instructions:
        if type(inst).__name__ == "InstTriggerDma":
            trig_inst = inst  # keep last
    prep_si = kv_prep.sync_info
    dve_waits = [
        w for w in prep_si.on_wait if w.ant_name and w.ant_name.startswith("DVE")
    ]
    assert len(dve_waits) == 1 and trig_inst is not None
    prep_si.on_wait = [
        w
        for w in prep_si.on_wait
        if not (w.ant_name and w.ant_name.startswith("DVE"))
    ]
    trig_si = trig_inst.sync_info
    trig_si.on_wait = list(trig_si.on_wait) + dve_waits

    # Strip DMA-completion waits that are already implied by same-engine
    # program order: the second DVE op of a 2-row chunk re-waits the same
    # semaphore threshold the first op already cleared.
    seen: set = set()
    for inst in prep_blk.instructions:
        if type(inst).__name__ != "InstCustomDveAnt":
            continue
        si = inst.sync_info
        if si is None:
            continue
        keep = []
        for w in si.on_wait:
            key = (w.id, w.wait_mode, w.wait_value)
            if key in seen:
                continue
            keep.append(w)
            seen.add(key)
        si.on_wait = keep

    nc.compile()
    return nc


def _pack31(bits):
    """bits [..., D] uint8 -> [..., WPK] int32 words, 31 payload bits per
    word at positions 0..29 and 31 (bit 30 always zero -> never NaN/Inf)."""
    lead = bits.shape[:-1]
    b32 = np.zeros(lead + (WPK, 32), dtype=np.uint8)
    pad = np.zeros(lead + (DPAD,), dtype=np.uint8)
    pad[..., :D] = bits
    pad = pad.reshape(lead + (WPK, PAYLOAD))
    b32[..., :30] = pad[..., :30]
    b32[..., 31] = pad[..., 30]
    words = np.packbits(b32.reshape(lead + (WPK * 32,)), axis=-1, bitorder="little")
    return words.reshape(lead + (WPK * 4,)).view(np.int32)


def _pack_inputs(x, bit_weights):
    x = np.asarray(x).astype(np.uint8)
    bw = np.ascontiguousarray(np.asarray(bit_weights).astype(np.uint8))
    notx = (1 - x).astype(np.uint8)
    nxp = _pack31(notx)  # [WPK]
    wp = _pack31(bw)  # [L, WPK]
    in_maps = []
    for i in range(NCORES):
        shard = wp[i * LSH : (i + 1) * LSH].reshape(128, NB, WPK)
        wx = np.empty((128, NB + 1, WPK), dtype=np.int32)
        wx[:, 0, :] = nxp
        wx[:, 1:, :] = shard
        in_maps.append({"wx": wx.reshape(128, (NB + 1) * WPK)})
    return in_maps


def _gather(results):
    outs = []
    for i in range(NCORES):
        # [1, 128, 1, NB] fp32 violation flags; res[0, p, 0, b] covers
        # neuron 8p + b; flag == 0.0 means no violated requirement -> True
        res = results[i]["res"].reshape(128, NB)
        outs.append(res.reshape(-1) == 0.0)
    return np.concatenate(outs).astype(np.bool_)


def _get_compiled():
    global _compiled
    if _compiled is None:
        _compiled = _build()
    return _compiled


def kernel(x, bit_weights):
    from concourse import bass_utils

    nc = _get_compiled()
    in_maps = _pack_inputs(x, bit_weights)
    r = bass_utils.run_bass_kernel_spmd(nc, in_maps, core_ids=list(range(NCORES)))
    return _gather(r.results)


# revision 45
# speedup vs baseline: 1.2486x; 1.0288x over previous
"""HardAndLayer on 8 Trainium2 NeuronCores.

out[l] = AND_d (x[d] OR NOT w[l,d])  ==  no d with (w[l,d] AND NOT x[d])

Strategy (per sharding hint): shard bit_weights row-wise (neuron dim) across
8 cores, x replicated, no collectives.

Wire format: bools are bit-packed 31 per int32 word with bit 30 forced
zero, so no word can form an fp32 NaN/Inf pattern (the DVE fp32 stream
path canonicalizes NaN operands — HW-verified failure with full 32-bit
packing). DRAM tensors are declared int32; on device the SBUF APs are
bitcast to fp32 for the custom DVE op, whose datapath is bitwise: per row
    acc[p] = fold_logical_or_j (w_packed[p, j] BITWISE_AND notx_packed[j])
(identity fp32 converter, BITWISE_AND preserves raw bits, LOGICAL_OR folds
on bit-pattern truthiness; -0.0/NaN/denormal patterns all count as nonzero).
out[l] = (acc == 0), applied on the host to the DMA'd per-neuron flags.

Pipeline: the weight shard streams in 4 HWDGE DMA chunks (each chunk->DVE
edge re-pays the ~900ns DMA-semaphore latency, so sizes are chosen to
keep the per-chunk anchors A_k = arrival_k + 900 + dve_work_after_k flat;
more chunks would serialize on the single 625ns/instr HWDGE). The result
write is a prepared SWDGE kv_writeback fired by trigger_dma right after
the last DVE accumulator write: its descriptors are generated early, off
the critical path, so the tail skips HWDGE descriptor-gen and the DGE
start delay entirely.

Layout: partition p of a core holds its 8 consecutive neuron rows
(8 KB contiguous per partition); res[p, b] = out[8p + b].
"""

import numpy as np

L = 8192
D = 8192
NCORES = 8
LSH = L // NCORES  # 1024 neuron rows per core
PAYLOAD = 31  # bits per packed word; bit 30 held zero -> never NaN/Inf.
# The DVE fp32 stream path canonicalizes NaN-pattern operands (HW-verified:
# 32-bit packing fails exactly on words with the exponent field all-ones),
# so the wire format must never form one. Denormals and -0.0 pass bit-exact.
WPK = -(-D // PAYLOAD)  # 265 packed words per neuron row
DPAD = WPK * PAYLOAD
NB = LSH // 128  # 8 neuron rows per partition
# Per-partition DRAM layout: [notx | row0 | ... | row7], 9*WPK words.
# Chunk sizes in row units (1 unit = WPK words = ~1 KiB/partition):
# (notx,r0,r1 | r2,r3 | r4,r5 | r6,r7) -- flat anchor schedule; 4 chunks
# keep the serial HWDGE descriptor-gen (625ns/instr) off the DMA stream.
CHUNK_UNITS = (3, 2, 2, 1, 1)
assert sum(CHUNK_UNITS) == NB + 1
CHUNK_COLS = tuple(u * WPK for u in CHUNK_UNITS)

_compiled = None
_custom_op = None


def _register_custom_op():
    """Register the fused AND+any op in the custom-DVE table (idempotent)."""
    global _custom_op
    if _custom_op is not None:
        return _custom_op
    from concourse import dve_ops
    from concourse.dve_spec import Spec, Src0, Src1, Zero, Bin, lower
    from concourse.dve_uop import AluOp, DveOpSpec

    name = "AND_ANY_ANT"
    for o in dve_ops.OPS:
        if o.name == name:
            _custom_op = o
            return o

    def _ref(in0, in1, c0, c1, c2):
        a = in0.view(np.uint32) & in1.view(np.uint32)
        acc = (
            (a.reshape(a.shape[0], -1) != 0)
            .any(axis=-1, keepdims=True)
            .astype(np.float32)
        )
        return a.view(np.float32), acc

    spec = Spec(
        body=Bin(AluOp.BITWISE_AND, Src0, Src1),
        accum=AluOp.LOGICAL_OR,
        accum_init=Zero,
        reference=_ref,
    )
    shas = {}
    for ver in ("v3", "v4"):
        try:
            uops = lower(spec, ver=ver)
            shas[ver] = DveOpSpec(name=name, uops=uops, rd1_en=True).sha(ver)
        except Exception:
            pass
    op = dve_ops.DveOp(name, spec, subdim=False, uops_sha=shas)
    dve_ops.OPS.append(op)
    dve_ops._SUB_OPCODE_FOR_NAME[name] = (
        dve_ops._CUSTOM_DVE_ROW_BASE + len(dve_ops.OPS) - 1
    )
    dve_ops.CUSTOM_DVE_SPECS[name] = spec
    _custom_op = op
    return op


def _build():
    import concourse.bacc as bacc
    import concourse.mybir as mybir
    from concourse import tile

    op = _register_custom_op()

    # Bass.__init__ registers 4 const APs via gpsimd.memset and emits a
    # kernel-start all-engine barrier. Nothing in this kernel reads the
    # const APs, and every cross-engine edge here is semaphore-gated (DMA
    # completion sems / engine ticks), so neither is needed; together they
    # delay the first DMA by ~700ns. Suppress both during construction.
    import concourse.bass as cbass

    orig_memset = cbass.BassGpSimd.memset
    orig_barrier = cbass.Bass.all_engine_barrier
    cbass.BassGpSimd.memset = lambda self, ap, constant: None
    cbass.Bass.all_engine_barrier = lambda self, *a, **k: None
    try:
        nc = bacc.Bacc(
            "TRN2",
            target_bir_lowering=False,
            debug=False,
            enable_asserts=False,
            num_devices=NCORES,
        )
    finally:
        cbass.BassGpSimd.memset = orig_memset
        cbass.Bass.all_engine_barrier = orig_barrier
    TOT = (NB + 1) * WPK
    wx = nc.dram_tensor("wx", [128, TOT], mybir.dt.int32, kind="ExternalInput")
    # kv_writeback-shaped result: [batch=1, dhi=128, dho=1, n_ctx=NB]
    res = nc.dram_tensor(
        "res", [1, 128, 1, NB], mybir.dt.float32, kind="ExternalOutput"
    )

    with tile.TileContext(nc) as tc:
        with (
            tc.tile_pool(name="wpool", bufs=1) as wpool,
            tc.tile_pool(name="small", bufs=1) as small,
        ):
            acc = small.tile([128, NB], mybir.dt.float32)
            idx = small.tile([128, 1], mybir.dt.int32)
            m = small.tile([128, WPK], mybir.dt.float32, tag="m")
            nc.gpsimd.memset(idx[:], 0)
            res_sem = nc.alloc_semaphore("res_dma_sem")

            # Weight + notx stream: HWDGE chunks, alternating SP/Act.
            tiles = []
            c0 = 0
            for ci, cw in enumerate(CHUNK_COLS):
                wt = wpool.tile([128, cw], mybir.dt.int32, tag=f"wt{ci}")
                dma_eng = nc.sync if ci % 2 == 0 else nc.scalar
                dma_eng.dma_start(wt[:], wx[:, c0 : c0 + cw])
                tiles.append((wt, c0, cw))
                c0 += cw

            def words_ap(col, n):
                """fp32-bitcast AP over words [col, col+n) of the stream."""
                for wt, tc0, tcw in tiles:
                    if tc0 <= col and col + n <= tc0 + tcw:
                        return wt[:, col - tc0 : col - tc0 + n].bitcast(
                            mybir.dt.float32
                        )
                raise AssertionError(f"span {col}+{n} crosses a chunk boundary")

            nx_ap = words_ap(0, WPK)  # notx is the stream head
            for gb in range(NB):
                nc.vector._custom_dve(
                    op,
                    out=m[:],
                    in0=words_ap((gb + 1) * WPK, WPK),
                    in1=nx_ap,
                    accum_out=acc[:, gb : gb + 1],
                )
            # Prepared result writeback: the prep only generates descriptors
            # (its RAW dep on `acc` is demoted to a no-sync edge, so the gen
            # runs early on the idle Pool engine); the transfer fires at
            # trigger_dma, which carries the sync deps on the accumulator
            # writes. The tail thus skips HWDGE desc-gen + DGE start delay.
            nc.gpsimd.kv_writeback(
                out_ap=res[:, :, :, :],
                in_ap=acc[:].rearrange("p (a b n) -> p a b n", a=1, b=1),
                ctx_idxs_ap=idx[:],
                prepare_only=True,
                sem=res_sem,
            )
            nc.gpsimd.trigger_dma(count=None)

    # Tile assigns each SWDGE prep a DMASW lane: consumers and the
    # end-of-kernel gather wait on DMASW<k> >= 16, but pass 2 leaves the
    # descriptor sem slot (on_update[0]) at our explicit `sem=`, which
    # nothing then waits on. Retarget each prep's descriptor sem to its
    # Tile-assigned lane semaphore (lanes are assigned to Pool DMA
    # instructions in program order) so the DMA completions tick the
    # lanes the waits actually watch.
    dmasw_ids = {}
    preps = []
    prep_blk = None
    for blk in nc.m.functions[0].blocks:
        for inst in blk.instructions:
            tn = type(inst).__name__
            if (
                tn in ("InstKVWritebackAnt", "InstDMAGatherAnt")
                and getattr(inst, "gen_mode", 0) == 1
            ):
                preps.append(inst)
                prep_blk = blk
            si = inst.sync_info
            if si is not None:
                for w in si.on_wait:
                    if w.ant_name and w.ant_name.startswith("DMASW"):
                        dmasw_ids[w.ant_name.split("_")[0]] = w.id
    assert len(preps) == 1 and dmasw_ids
    nlanes = len(dmasw_ids)
    for k, prep in enumerate(preps):
        upd = prep.sync_info.on_update
        assert upd[0].ant_name in ("nx_dma_sem", "res_dma_sem")
        upd[0].id = dmasw_ids[f"DMASW{k % nlanes}"]
        prep.sync_info.on_update = upd

    # Tile attaches the result-trigger's data dep (all DVE accumulator
    # writes) to the kv_writeback PREP, which would push the ~1us
    # descriptor-gen onto the post-compute critical path. Desc-gen reads
    # no tensor data, so move the DVE wait from the prep to the final
    # trigger: gen runs early on the idle Pool engine and only the
    # transfer launch gates on the DVE ops.
    kv_prep = preps[0]
    assert type(kv_prep).__name__ == "InstKVWritebackAnt"
    trig_inst = None
    for inst in prep_blk.instructions:
        if type(inst).__name__ == "InstTriggerDma":
            trig_inst = inst  # keep last
    prep_si = kv_prep.sync_info
    dve_waits = [
        w for w in prep_si.on_wait if w.ant_name and w.ant_name.startswith("DVE")
    ]
    assert len(dve_waits) == 1 and trig_inst is not None
    prep_si.on_wait = [
        w
        for w in prep_si.on_wait
        if not (w.ant_name and w.ant_name.startswith("DVE"))
    ]
    trig_si = trig_inst.sync_info
    trig_si.on_wait = list(trig_si.on_wait) + dve_waits

    # Strip DMA-completion waits that are already implied by same-engine
    # program order: the second DVE op of a 2-row chunk re-waits the same
    # semaphore threshold the first op already cleared.
    seen: set = set()
    for inst in prep_blk.instructions:
        if type(inst).__name__ != "InstCustomDveAnt":
            continue
        si = inst.sync_info
        if si is None:
            continue
        keep = []
        for w in si.on_wait:
            key = (w.id, w.wait_mode, w.wait_value)
            if key in seen:
                continue
            keep.append(w)
            seen.add(key)
        si.on_wait = keep

    # The epilogue runs TWO gather/release barrier rounds around the
    # semaphore range-clear. Round 2 is pure exit synchronization: every
    # engine already drained in round 1, the barrier sems are balanced at
    # zero after round 1, and program completion waits for all engines
    # regardless. Delete round 2 (everything after the clear), keeping one
    # Pool drain so the clear's engine op is flushed before Pool's stream
    # ends. The DMA-wait -> round-1 -> clear ordering that protects
    # cross-invocation semaphore state is untouched.
    end_blk = next(b for b in nc.m.functions[0].blocks if b.name.endswith("_end"))
    insts = end_blk.instructions
    clear_i = next(
        i
        for i, x in enumerate(insts)
        if getattr(x, "op_name", None) == "EVENT_SEMAPHORE_RANGE_CLEAR"
    )
    tail = list(insts[clear_i + 1 :])
    keep_drain = next(
        x
        for x in tail
        if type(x).__name__ == "InstDrain" and str(x.engine).endswith("Pool")
    )
    for x in tail:
        if x is not keep_drain:
            insts.remove(x)

    nc.compile()
    return nc


def _pack31(bits):
    """bits [..., D] uint8 -> [..., WPK] int32 words, 31 payload bits per
    word at positions 0..29 and 31 (bit 30 always zero -> never NaN/Inf)."""
    lead = bits.shape[:-1]
    b32 = np.zeros(lead + (WPK, 32), dtype=np.uint8)
    pad = np.zeros(lead + (DPAD,), dtype=np.uint8)
    pad[..., :D] = bits
    pad = pad.reshape(lead + (WPK, PAYLOAD))
    b32[..., :30] = pad[..., :30]
    b32[..., 31] = pad[..., 30]
    words = np.packbits(b32.reshape(lead + (WPK * 32,)), axis=-1, bitorder="little")
    return words.reshape(lead + (WPK * 4,)).view(np.int32)


def _pack_inputs(x, bit_weights):
    x = np.asarray(x).astype(np.uint8)
    bw = np.ascontiguousarray(np.asarray(bit_weights).astype(np.uint8))
    notx = (1 - x).astype(np.uint8)
    nxp = _pack31(notx)  # [WPK]
    wp = _pack31(bw)  # [L, WPK]
    in_maps = []
    for i in range(NCORES):
        shard = wp[i * LSH : (i + 1) * LSH].reshape(128, NB, WPK)
        wx = np.empty((128, NB + 1, WPK), dtype=np.int32)
        wx[:, 0, :] = nxp
        wx[:, 1:, :] = shard
        in_maps.append({"wx": wx.reshape(128, (NB + 1) * WPK)})
    return in_maps


def _gather(results):
    outs = []
    for i in range(NCORES):
        # [1, 128, 1, NB] fp32 violation flags; res[0, p, 0, b] covers
        # neuron 8p + b; flag == 0.0 means no violated requirement -> True
        res = results[i]["res"].reshape(128, NB)
        outs.append(res.reshape(-1) == 0.0)
    return np.concatenate(outs).astype(np.bool_)


def _get_compiled():
    global _compiled
    if _compiled is None:
        _compiled = _build()
    return _compiled


def kernel(x, bit_weights):
    from concourse import bass_utils

    nc = _get_compiled()
    in_maps = _pack_inputs(x, bit_weights)
    r = bass_utils.run_bass_kernel_spmd(nc, in_maps, core_ids=list(range(NCORES)))
    return _gather(r.results)


# revision 48
# speedup vs baseline: 1.2717x; 1.0185x over previous
"""HardAndLayer on 8 Trainium2 NeuronCores.

out[l] = AND_d (x[d] OR NOT w[l,d])  ==  no d with (w[l,d] AND NOT x[d])

Strategy (per sharding hint): shard bit_weights row-wise (neuron dim) across
8 cores, x replicated, no collectives.

Wire format: bools are bit-packed 31 per int32 word with bit 30 forced
zero, so no word can form an fp32 NaN/Inf pattern (the DVE fp32 stream
path canonicalizes NaN operands — HW-verified failure with full 32-bit
packing). DRAM tensors are declared int32; on device the SBUF APs are
bitcast to fp32 for the custom DVE op, whose datapath is bitwise: per row
    acc[p] = fold_logical_or_j (w_packed[p, j] BITWISE_AND notx_packed[j])
(identity fp32 converter, BITWISE_AND preserves raw bits, LOGICAL_OR folds
on bit-pattern truthiness; -0.0/NaN/denormal patterns all count as nonzero).
out[l] = (acc == 0), applied on the host to the DMA'd per-neuron flags.

Pipeline: the weight shard streams in 4 HWDGE DMA chunks (each chunk->DVE
edge re-pays the ~900ns DMA-semaphore latency, so sizes are chosen to
keep the per-chunk anchors A_k = arrival_k + 900 + dve_work_after_k flat;
more chunks would serialize on the single 625ns/instr HWDGE). The result
write is a prepared SWDGE kv_writeback fired by trigger_dma right after
the last DVE accumulator write: its descriptors are generated early, off
the critical path, so the tail skips HWDGE descriptor-gen and the DGE
start delay entirely.

Layout: partition p of a core holds its 8 consecutive neuron rows
(8 KB contiguous per partition); res[p, b] = out[8p + b].
"""

import numpy as np

L = 8192
D = 8192
NCORES = 8
LSH = L // NCORES  # 1024 neuron rows per core
PAYLOAD = 31  # bits per packed word; bit 30 held zero -> never NaN/Inf.
# The DVE fp32 stream path canonicalizes NaN-pattern operands (HW-verified:
# 32-bit packing fails exactly on words with the exponent field all-ones),
# so the wire format must never form one. Denormals and -0.0 pass bit-exact.
WPK = -(-D // PAYLOAD)  # 265 packed words per neuron row
DPAD = WPK * PAYLOAD
NB = LSH // 128  # 8 neuron rows per partition
# Per-partition DRAM layout: [notx | row0 | ... | row7], 9*WPK words.
# Chunk sizes in row units (1 unit = WPK words = ~1 KiB/partition):
# (notx,r0,r1 | r2,r3 | r4,r5 | r6,r7) -- flat anchor schedule; 4 chunks
# keep the serial HWDGE descriptor-gen (625ns/instr) off the DMA stream.
CHUNK_UNITS = (3, 2, 2, 1, 1)
assert sum(CHUNK_UNITS) == NB + 1
CHUNK_COLS = tuple(u * WPK for u in CHUNK_UNITS)

_compiled = None
_custom_op = None


def _register_custom_op():
    """Register the fused AND+any op in the custom-DVE table (idempotent)."""
    global _custom_op
    if _custom_op is not None:
        return _custom_op
    from concourse import dve_ops
    from concourse.dve_spec import Spec, Src0, Src1, Zero, Bin, lower
    from concourse.dve_uop import AluOp, DveOpSpec

    name = "AND_ANY_ANT"
    for o in dve_ops.OPS:
        if o.name == name:
            _custom_op = o
            return o

    def _ref(in0, in1, c0, c1, c2):
        a = in0.view(np.uint32) & in1.view(np.uint32)
        acc = (
            (a.reshape(a.shape[0], -1) != 0)
            .any(axis=-1, keepdims=True)
            .astype(np.float32)
        )
        return a.view(np.float32), acc

    spec = Spec(
        body=Bin(AluOp.BITWISE_AND, Src0, Src1),
        accum=AluOp.LOGICAL_OR,
        accum_init=Zero,
        reference=_ref,
    )
    shas = {}
    for ver in ("v3", "v4"):
        try:
            uops = lower(spec, ver=ver)
            shas[ver] = DveOpSpec(name=name, uops=uops, rd1_en=True).sha(ver)
        except Exception:
            pass
    op = dve_ops.DveOp(name, spec, subdim=False, uops_sha=shas)
    dve_ops.OPS.append(op)
    dve_ops._SUB_OPCODE_FOR_NAME[name] = (
        dve_ops._CUSTOM_DVE_ROW_BASE + len(dve_ops.OPS) - 1
    )
    dve_ops.CUSTOM_DVE_SPECS[name] = spec
    _custom_op = op
    return op


def _build():
    import concourse.bacc as bacc
    import concourse.mybir as mybir
    from concourse import tile

    op = _register_custom_op()

    # Bass.__init__ registers 4 const APs via gpsimd.memset and emits a
    # kernel-start all-engine barrier. Nothing in this kernel reads the
    # const APs, and every cross-engine edge here is semaphore-gated (DMA
    # completion sems / engine ticks), so neither is needed; together they
    # delay the first DMA by ~700ns. Suppress both during construction.
    import concourse.bass as cbass

    orig_memset = cbass.BassGpSimd.memset
    orig_barrier = cbass.Bass.all_engine_barrier
    cbass.BassGpSimd.memset = lambda self, ap, constant: None
    cbass.Bass.all_engine_barrier = lambda self, *a, **k: None
    try:
        nc = bacc.Bacc(
            "TRN2",
            target_bir_lowering=False,
            debug=False,
            enable_asserts=False,
            num_devices=NCORES,
        )
    finally:
        cbass.BassGpSimd.memset = orig_memset
        cbass.Bass.all_engine_barrier = orig_barrier
    TOT = (NB + 1) * WPK
    wx = nc.dram_tensor("wx", [128, TOT], mybir.dt.int32, kind="ExternalInput")
    # kv_writeback-shaped result: [batch=1, dhi=128, dho=1, n_ctx=NB]
    res = nc.dram_tensor(
        "res", [1, 128, 1, NB], mybir.dt.float32, kind="ExternalOutput"
    )

    with tile.TileContext(nc) as tc:
        with (
            tc.tile_pool(name="wpool", bufs=1) as wpool,
            tc.tile_pool(name="small", bufs=1) as small,
        ):
            acc = small.tile([128, NB], mybir.dt.float32)
            idx = small.tile([128, 1], mybir.dt.int32)
            m = small.tile([128, WPK], mybir.dt.float32, tag="m")
            nc.gpsimd.memset(idx[:], 0)
            res_sem = nc.alloc_semaphore("res_dma_sem")

            # Weight + notx stream: HWDGE chunks, alternating SP/Act.
            tiles = []
            c0 = 0
            for ci, cw in enumerate(CHUNK_COLS):
                wt = wpool.tile([128, cw], mybir.dt.int32, tag=f"wt{ci}")
                dma_eng = nc.sync if ci % 2 == 0 else nc.scalar
                dma_eng.dma_start(wt[:], wx[:, c0 : c0 + cw])
                tiles.append((wt, c0, cw))
                c0 += cw

            def words_ap(col, n):
                """fp32-bitcast AP over words [col, col+n) of the stream."""
                for wt, tc0, tcw in tiles:
                    if tc0 <= col and col + n <= tc0 + tcw:
                        return wt[:, col - tc0 : col - tc0 + n].bitcast(
                            mybir.dt.float32
                        )
                raise AssertionError(f"span {col}+{n} crosses a chunk boundary")

            nx_ap = words_ap(0, WPK)  # notx is the stream head
            for gb in range(NB):
                nc.vector._custom_dve(
                    op,
                    out=m[:],
                    in0=words_ap((gb + 1) * WPK, WPK),
                    in1=nx_ap,
                    accum_out=acc[:, gb : gb + 1],
                )
            # Prepared result writeback: the prep only generates descriptors
            # (its RAW dep on `acc` is demoted to a no-sync edge, so the gen
            # runs early on the idle Pool engine); the transfer fires at
            # trigger_dma, which carries the sync deps on the accumulator
            # writes. The tail thus skips HWDGE desc-gen + DGE start delay.
            nc.gpsimd.kv_writeback(
                out_ap=res[:, :, :, :],
                in_ap=acc[:].rearrange("p (a b n) -> p a b n", a=1, b=1),
                ctx_idxs_ap=idx[:],
                prepare_only=True,
                sem=res_sem,
            )
            nc.gpsimd.trigger_dma(count=None)

    # Tile assigns each SWDGE prep a DMASW lane: consumers and the
    # end-of-kernel gather wait on DMASW<k> >= 16, but pass 2 leaves the
    # descriptor sem slot (on_update[0]) at our explicit `sem=`, which
    # nothing then waits on. Retarget each prep's descriptor sem to its
    # Tile-assigned lane semaphore (lanes are assigned to Pool DMA
    # instructions in program order) so the DMA completions tick the
    # lanes the waits actually watch.
    dmasw_ids = {}
    preps = []
    prep_blk = None
    for blk in nc.m.functions[0].blocks:
        for inst in blk.instructions:
            tn = type(inst).__name__
            if (
                tn in ("InstKVWritebackAnt", "InstDMAGatherAnt")
                and getattr(inst, "gen_mode", 0) == 1
            ):
                preps.append(inst)
                prep_blk = blk
            si = inst.sync_info
            if si is not None:
                for w in si.on_wait:
                    if w.ant_name and w.ant_name.startswith("DMASW"):
                        dmasw_ids[w.ant_name.split("_")[0]] = w.id
    assert len(preps) == 1 and dmasw_ids
    nlanes = len(dmasw_ids)
    for k, prep in enumerate(preps):
        upd = prep.sync_info.on_update
        assert upd[0].ant_name in ("nx_dma_sem", "res_dma_sem")
        upd[0].id = dmasw_ids[f"DMASW{k % nlanes}"]
        prep.sync_info.on_update = upd

    # Tile attaches the result-trigger's data dep (all DVE accumulator
    # writes) to the kv_writeback PREP, which would push the ~1us
    # descriptor-gen onto the post-compute critical path. Desc-gen reads
    # no tensor data, so move the DVE wait from the prep to the final
    # trigger: gen runs early on the idle Pool engine and only the
    # transfer launch gates on the DVE ops.
    kv_prep = preps[0]
    assert type(kv_prep).__name__ == "InstKVWritebackAnt"
    trig_inst = None
    for inst in prep_blk.instructions:
        if type(inst).__name__ == "InstTriggerDma":
            trig_inst = inst  # keep last
    prep_si = kv_prep.sync_info
    dve_waits = [
        w for w in prep_si.on_wait if w.ant_name and w.ant_name.startswith("DVE")
    ]
    assert len(dve_waits) == 1 and trig_inst is not None
    prep_si.on_wait = [
        w
        for w in prep_si.on_wait
        if not (w.ant_name and w.ant_name.startswith("DVE"))
    ]
    trig_si = trig_inst.sync_info
    trig_si.on_wait = list(trig_si.on_wait) + dve_waits

    # Strip DMA-completion waits that are already implied by same-engine
    # program order: the second DVE op of a 2-row chunk re-waits the same
    # semaphore threshold the first op already cleared.
    seen: set = set()
    for inst in prep_blk.instructions:
        if type(inst).__name__ != "InstCustomDveAnt":
            continue
        si = inst.sync_info
        if si is None:
            continue
        keep = []
        for w in si.on_wait:
            key = (w.id, w.wait_mode, w.wait_value)
            if key in seen:
                continue
            keep.append(w)
            seen.add(key)
        si.on_wait = keep

    # The epilogue runs TWO gather/release barrier rounds around the
    # semaphore range-clear. Round 2 is pure exit synchronization: every
    # engine already drained in round 1, the barrier sems are balanced at
    # zero after round 1, and program completion waits for all engines
    # regardless. Delete round 2 (everything after the clear), keeping one
    # Pool drain so the clear's engine op is flushed before Pool's stream
    # ends. The DMA-wait -> round-1 -> clear ordering that protects
    # cross-invocation semaphore state is untouched.
    end_blk = next(b for b in nc.m.functions[0].blocks if b.name.endswith("_end"))
    insts = end_blk.instructions
    clear_i = next(
        i
        for i, x in enumerate(insts)
        if getattr(x, "op_name", None) == "EVENT_SEMAPHORE_RANGE_CLEAR"
    )
    tail = list(insts[clear_i + 1 :])
    keep_drain = next(
        x
        for x in tail
        if type(x).__name__ == "InstDrain" and str(x.engine).endswith("Pool")
    )
    for x in tail:
        if x is not keep_drain:
            insts.remove(x)

    # The result-DMA completion wait sits on SP's end-drain, so after the
    # sem fires the path still runs SP-drain -> gather-inc -> sem-prop
    # before Pool (who must order the clear after DMA completion) can
    # proceed. Move the DMASW wait onto Pool's round-1 gather directly:
    # SP increments the gather early and Pool's gather carries both
    # conditions, deleting the SP hop from the critical path. Increment/
    # subtract arithmetic is unchanged -- only wait placement moves.
    sp_drain = next(
        x
        for x in end_blk.instructions
        if type(x).__name__ == "InstDrain"
        and str(x.engine).endswith("SP")
        and x.sync_info is not None
        and any(
            w.ant_name and w.ant_name.startswith("DMASW")
            for w in x.sync_info.on_wait
        )
    )
    pool_gather = next(
        x
        for x in end_blk.instructions
        if type(x).__name__ == "InstEventSemaphore"
        and str(x.engine).endswith("Pool")
        and x.sync_info is not None
        and any(
            w.ant_name and "gather" in w.ant_name for w in x.sync_info.on_wait
        )
    )
    sp_si = sp_drain.sync_info
    moved = [
        w for w in sp_si.on_wait if w.ant_name and w.ant_name.startswith("DMASW")
    ]
    assert len(moved) == 1
    sp_si.on_wait = [
        w
        for w in sp_si.on_wait
        if not (w.ant_name and w.ant_name.startswith("DMASW"))
    ]
    pg_si = pool_gather.sync_info
    pg_si.on_wait = list(pg_si.on_wait) + moved

    nc.compile()
    return nc


def _pack31(bits):
    """bits [..., D] uint8 -> [..., WPK] int32 words, 31 payload bits per
    word at positions 0..29 and 31 (bit 30 always zero -> never NaN/Inf)."""
    lead = bits.shape[:-1]
    b32 = np.zeros(lead + (WPK, 32), dtype=np.uint8)
    pad = np.zeros(lead + (DPAD,), dtype=np.uint8)
    pad[..., :D] = bits
    pad = pad.reshape(lead + (WPK, PAYLOAD))
    b32[..., :30] = pad[..., :30]
    b32[..., 31] = pad[..., 30]
    words = np.packbits(b32.reshape(lead + (WPK * 32,)), axis=-1, bitorder="little")
    return words.reshape(lead + (WPK * 4,)).view(np.int32)


def _pack_inputs(x, bit_weights):
    x = np.asarray(x).astype(np.uint8)
    bw = np.ascontiguousarray(np.asarray(bit_weights).astype(np.uint8))
    notx = (1 - x).astype(np.uint8)
    nxp = _pack31(notx)  # [WPK]
    wp = _pack31(bw)  # [L, WPK]
    in_maps = []
    for i in range(NCORES):
        shard = wp[i * LSH : (i + 1) * LSH].reshape(128, NB, WPK)
        wx = np.empty((128, NB + 1, WPK), dtype=np.int32)
        wx[:, 0, :] = nxp
        wx[:, 1:, :] = shard
        in_maps.append({"wx": wx.reshape(128, (NB + 1) * WPK)})
    return in_maps


def _gather(results):
    outs = []
    for i in range(NCORES):
        # [1, 128, 1, NB] fp32 violation flags; res[0, p, 0, b] covers
        # neuron 8p + b; flag == 0.0 means no violated requirement -> True
        res = results[i]["res"].reshape(128, NB)
        outs.append(res.reshape(-1) == 0.0)
    return np.concatenate(outs).astype(np.bool_)


def _get_compiled():
    global _compiled
    if _compiled is None:
        _compiled = _build()
    return _compiled


def kernel(x, bit_weights):
    from concourse import bass_utils

    nc = _get_compiled()
    in_maps = _pack_inputs(x, bit_weights)
    r = bass_utils.run_bass_kernel_spmd(nc, in_maps, core_ids=list(range(NCORES)))
    return _gather(r.results)


# revision 49
# speedup vs baseline: 1.2819x; 1.0080x over previous
"""HardAndLayer on 8 Trainium2 NeuronCores.

out[l] = AND_d (x[d] OR NOT w[l,d])  ==  no d with (w[l,d] AND NOT x[d])

Strategy (per sharding hint): shard bit_weights row-wise (neuron dim) across
8 cores, x replicated, no collectives.

Wire format: bools are bit-packed 31 per int32 word with bit 30 forced
zero, so no word can form an fp32 NaN/Inf pattern (the DVE fp32 stream
path canonicalizes NaN operands — HW-verified failure with full 32-bit
packing). DRAM tensors are declared int32; on device the SBUF APs are
bitcast to fp32 for the custom DVE op, whose datapath is bitwise: per row
    acc[p] = fold_logical_or_j (w_packed[p, j] BITWISE_AND notx_packed[j])
(identity fp32 converter, BITWISE_AND preserves raw bits, LOGICAL_OR folds
on bit-pattern truthiness; -0.0/NaN/denormal patterns all count as nonzero).
out[l] = (acc == 0), applied on the host to the DMA'd per-neuron flags.

Pipeline: the weight shard streams in 4 HWDGE DMA chunks (each chunk->DVE
edge re-pays the ~900ns DMA-semaphore latency, so sizes are chosen to
keep the per-chunk anchors A_k = arrival_k + 900 + dve_work_after_k flat;
more chunks would serialize on the single 625ns/instr HWDGE). The result
write is a prepared SWDGE kv_writeback fired by trigger_dma right after
the last DVE accumulator write: its descriptors are generated early, off
the critical path, so the tail skips HWDGE descriptor-gen and the DGE
start delay entirely.

Layout: partition p of a core holds its 8 consecutive neuron rows
(8 KB contiguous per partition); res[p, b] = out[8p + b].
"""

import numpy as np

L = 8192
D = 8192
NCORES = 8
LSH = L // NCORES  # 1024 neuron rows per core
PAYLOAD = 31  # bits per packed word; bit 30 held zero -> never NaN/Inf.
# The DVE fp32 stream path canonicalizes NaN-pattern operands (HW-verified:
# 32-bit packing fails exactly on words with the exponent field all-ones),
# so the wire format must never form one. Denormals and -0.0 pass bit-exact.
WPK = -(-D // PAYLOAD)  # 265 packed words per neuron row
DPAD = WPK * PAYLOAD
NB = LSH // 128  # 8 neuron rows per partition
# Per-partition DRAM layout: [notx | row0 | ... | row7], 9*WPK words.
# Chunk sizes in row units (1 unit = WPK words = ~1 KiB/partition):
# (notx,r0,r1 | r2,r3 | r4,r5 | r6,r7) -- flat anchor schedule; 4 chunks
# keep the serial HWDGE descriptor-gen (625ns/instr) off the DMA stream.
CHUNK_UNITS = (3, 2, 2, 1, 1)
assert sum(CHUNK_UNITS) == NB + 1
CHUNK_COLS = tuple(u * WPK for u in CHUNK_UNITS)

_compiled = None
_custom_op = None


def _register_custom_op():
    """Register the fused AND+any op in the custom-DVE table (idempotent)."""
    global _custom_op
    if _custom_op is not None:
        return _custom_op
    from concourse import dve_ops
    from concourse.dve_spec import Spec, Src0, Src1, Zero, Bin, lower
    from concourse.dve_uop import AluOp, DveOpSpec

    name = "AND_ANY_ANT"
    for o in dve_ops.OPS:
        if o.name == name:
            _custom_op = o
            return o

    def _ref(in0, in1, c0, c1, c2):
        a = in0.view(np.uint32) & in1.view(np.uint32)
        acc = (
            (a.reshape(a.shape[0], -1) != 0)
            .any(axis=-1, keepdims=True)
            .astype(np.float32)
        )
        return a.view(np.float32), acc

    spec = Spec(
        body=Bin(AluOp.BITWISE_AND, Src0, Src1),
        accum=AluOp.LOGICAL_OR,
        accum_init=Zero,
        reference=_ref,
    )
    shas = {}
    for ver in ("v3", "v4"):
        try:
            uops = lower(spec, ver=ver)
            shas[ver] = DveOpSpec(name=name, uops=uops, rd1_en=True).sha(ver)
        except Exception:
            pass
    op = dve_ops.DveOp(name, spec, subdim=False, uops_sha=shas)
    dve_ops.OPS.append(op)
    dve_ops._SUB_OPCODE_FOR_NAME[name] = (
        dve_ops._CUSTOM_DVE_ROW_BASE + len(dve_ops.OPS) - 1
    )
    dve_ops.CUSTOM_DVE_SPECS[name] = spec
    _custom_op = op
    return op


def _build():
    import concourse.bacc as bacc
    import concourse.mybir as mybir
    from concourse import tile

    op = _register_custom_op()

    # Bass.__init__ registers 4 const APs via gpsimd.memset and emits a
    # kernel-start all-engine barrier. Nothing in this kernel reads the
    # const APs, and every cross-engine edge here is semaphore-gated (DMA
    # completion sems / engine ticks), so neither is needed; together they
    # delay the first DMA by ~700ns. Suppress both during construction.
    import concourse.bass as cbass

    orig_memset = cbass.BassGpSimd.memset
    orig_barrier = cbass.Bass.all_engine_barrier
    cbass.BassGpSimd.memset = lambda self, ap, constant: None
    cbass.Bass.all_engine_barrier = lambda self, *a, **k: None
    try:
        nc = bacc.Bacc(
            "TRN2",
            target_bir_lowering=False,
            debug=False,
            enable_asserts=False,
            num_devices=NCORES,
        )
    finally:
        cbass.BassGpSimd.memset = orig_memset
        cbass.Bass.all_engine_barrier = orig_barrier
    TOT = (NB + 1) * WPK
    wx = nc.dram_tensor("wx", [128, TOT], mybir.dt.int32, kind="ExternalInput")
    # kv_writeback-shaped result: [batch=1, dhi=128, dho=1, n_ctx=NB]
    res = nc.dram_tensor(
        "res", [1, 128, 1, NB], mybir.dt.float32, kind="ExternalOutput"
    )

    with tile.TileContext(nc) as tc:
        with (
            tc.tile_pool(name="wpool", bufs=1) as wpool,
            tc.tile_pool(name="small", bufs=1) as small,
        ):
            acc = small.tile([128, NB], mybir.dt.float32)
            idx = small.tile([128, 1], mybir.dt.int32)
            m = small.tile([128, WPK], mybir.dt.float32, tag="m")
            nc.gpsimd.memset(idx[:], 0)
            res_sem = nc.alloc_semaphore("res_dma_sem")

            # Weight + notx stream: HWDGE chunks, alternating SP/Act.
            tiles = []
            c0 = 0
            for ci, cw in enumerate(CHUNK_COLS):
                wt = wpool.tile([128, cw], mybir.dt.int32, tag=f"wt{ci}")
                dma_eng = nc.sync if ci % 2 == 0 else nc.scalar
                dma_eng.dma_start(wt[:], wx[:, c0 : c0 + cw])
                tiles.append((wt, c0, cw))
                c0 += cw

            def words_ap(col, n):
                """fp32-bitcast AP over words [col, col+n) of the stream."""
                for wt, tc0, tcw in tiles:
                    if tc0 <= col and col + n <= tc0 + tcw:
                        return wt[:, col - tc0 : col - tc0 + n].bitcast(
                            mybir.dt.float32
                        )
                raise AssertionError(f"span {col}+{n} crosses a chunk boundary")

            nx_ap = words_ap(0, WPK)  # notx is the stream head
            for gb in range(NB):
                nc.vector._custom_dve(
                    op,
                    out=m[:],
                    in0=words_ap((gb + 1) * WPK, WPK),
                    in1=nx_ap,
                    accum_out=acc[:, gb : gb + 1],
                )
            # Prepared result writeback: the prep only generates descriptors
            # (its RAW dep on `acc` is demoted to a no-sync edge, so the gen
            # runs early on the idle Pool engine); the transfer fires at
            # trigger_dma, which carries the sync deps on the accumulator
            # writes. The tail thus skips HWDGE desc-gen + DGE start delay.
            nc.gpsimd.kv_writeback(
                out_ap=res[:, :, :, :],
                in_ap=acc[:].rearrange("p (a b n) -> p a b n", a=1, b=1),
                ctx_idxs_ap=idx[:],
                prepare_only=True,
                sem=res_sem,
            )
            nc.gpsimd.trigger_dma(count=None)

    # Tile assigns each SWDGE prep a DMASW lane: consumers and the
    # end-of-kernel gather wait on DMASW<k> >= 16, but pass 2 leaves the
    # descriptor sem slot (on_update[0]) at our explicit `sem=`, which
    # nothing then waits on. Retarget each prep's descriptor sem to its
    # Tile-assigned lane semaphore (lanes are assigned to Pool DMA
    # instructions in program order) so the DMA completions tick the
    # lanes the waits actually watch.
    dmasw_ids = {}
    preps = []
    prep_blk = None
    for blk in nc.m.functions[0].blocks:
        for inst in blk.instructions:
            tn = type(inst).__name__
            if (
                tn in ("InstKVWritebackAnt", "InstDMAGatherAnt")
                and getattr(inst, "gen_mode", 0) == 1
            ):
                preps.append(inst)
                prep_blk = blk
            si = inst.sync_info
            if si is not None:
                for w in si.on_wait:
                    if w.ant_name and w.ant_name.startswith("DMASW"):
                        dmasw_ids[w.ant_name.split("_")[0]] = w.id
    assert len(preps) == 1 and dmasw_ids
    nlanes = len(dmasw_ids)
    for k, prep in enumerate(preps):
        upd = prep.sync_info.on_update
        assert upd[0].ant_name in ("nx_dma_sem", "res_dma_sem")
        upd[0].id = dmasw_ids[f"DMASW{k % nlanes}"]
        prep.sync_info.on_update = upd

    # Tile attaches the result-trigger's data dep (all DVE accumulator
    # writes) to the kv_writeback PREP, which would push the ~1us
    # descriptor-gen onto the post-compute critical path. Desc-gen reads
    # no tensor data, so move the DVE wait from the prep to the final
    # trigger: gen runs early on the idle Pool engine and only the
    # transfer launch gates on the DVE ops.
    kv_prep = preps[0]
    assert type(kv_prep).__name__ == "InstKVWritebackAnt"
    trig_inst = None
    for inst in prep_blk.instructions:
        if type(inst).__name__ == "InstTriggerDma":
            trig_inst = inst  # keep last
    prep_si = kv_prep.sync_info
    dve_waits = [
        w for w in prep_si.on_wait if w.ant_name and w.ant_name.startswith("DVE")
    ]
    assert len(dve_waits) == 1 and trig_inst is not None
    prep_si.on_wait = [
        w
        for w in prep_si.on_wait
        if not (w.ant_name and w.ant_name.startswith("DVE"))
    ]
    trig_si = trig_inst.sync_info
    trig_si.on_wait = list(trig_si.on_wait) + dve_waits

    # Strip DMA-completion waits that are already implied by same-engine
    # program order: the second DVE op of a 2-row chunk re-waits the same
    # semaphore threshold the first op already cleared.
    seen: set = set()
    for inst in prep_blk.instructions:
        if type(inst).__name__ != "InstCustomDveAnt":
            continue
        si = inst.sync_info
        if si is None:
            continue
        keep = []
        for w in si.on_wait:
            key = (w.id, w.wait_mode, w.wait_value)
            if key in seen:
                continue
            keep.append(w)
            seen.add(key)
        si.on_wait = keep

    # The epilogue runs TWO gather/release barrier rounds around the
    # semaphore range-clear. Round 2 is pure exit synchronization: every
    # engine already drained in round 1, the barrier sems are balanced at
    # zero after round 1, and program completion waits for all engines
    # regardless. Delete round 2 (everything after the clear), keeping one
    # Pool drain so the clear's engine op is flushed before Pool's stream
    # ends. The DMA-wait -> round-1 -> clear ordering that protects
    # cross-invocation semaphore state is untouched.
    end_blk = next(b for b in nc.m.functions[0].blocks if b.name.endswith("_end"))
    insts = end_blk.instructions
    clear_i = next(
        i
        for i, x in enumerate(insts)
        if getattr(x, "op_name", None) == "EVENT_SEMAPHORE_RANGE_CLEAR"
    )
    tail = list(insts[clear_i + 1 :])
    keep_drain = next(
        x
        for x in tail
        if type(x).__name__ == "InstDrain" and str(x.engine).endswith("Pool")
    )
    for x in tail:
        if x is not keep_drain:
            insts.remove(x)

    # The result-DMA completion wait sits on SP's end-drain, so after the
    # sem fires the path still runs SP-drain -> gather-inc -> sem-prop
    # before Pool (who must order the clear after DMA completion) can
    # proceed. Move the DMASW wait onto Pool's round-1 gather directly:
    # SP increments the gather early and Pool's gather carries both
    # conditions, deleting the SP hop from the critical path. Increment/
    # subtract arithmetic is unchanged -- only wait placement moves.
    sp_drain = next(
        x
        for x in end_blk.instructions
        if type(x).__name__ == "InstDrain"
        and str(x.engine).endswith("SP")
        and x.sync_info is not None
        and any(
            w.ant_name and w.ant_name.startswith("DMASW")
            for w in x.sync_info.on_wait
        )
    )
    pool_gather = next(
        x
        for x in end_blk.instructions
        if type(x).__name__ == "InstEventSemaphore"
        and str(x.engine).endswith("Pool")
        and x.sync_info is not None
        and any(
            w.ant_name and "gather" in w.ant_name for w in x.sync_info.on_wait
        )
    )
    sp_si = sp_drain.sync_info
    moved = [
        w for w in sp_si.on_wait if w.ant_name and w.ant_name.startswith("DMASW")
    ]
    assert len(moved) == 1
    sp_si.on_wait = [
        w
        for w in sp_si.on_wait
        if not (w.ant_name and w.ant_name.startswith("DMASW"))
    ]
    pg_si = pool_gather.sync_info
    pg_si.on_wait = list(pg_si.on_wait) + moved

    # With round 2 deleted, the release half of the round-1 handshake is
    # vestigial: engines would only pause at exit for Pool's release-add,
    # and the add sits serially on Pool's chain ahead of the clear.
    # Delete the release-add and the engines' release>=1 waits; the
    # release semaphore is then never touched (balanced at zero), and the
    # gather>=4 condition alone still orders the clear after all engines.
    vestigial = [
        x
        for x in list(end_blk.instructions)
        if type(x).__name__ == "InstEventSemaphore"
        and x.sync_info is not None
        and (
            any(
                w.ant_name and "release" in w.ant_name and w.wait_value
                for w in x.sync_info.on_wait
            )
            or any(
                u.ant_name and "release" in u.ant_name
                for u in x.sync_info.on_update
            )
        )
        and not any(
            w.ant_name and "gather" in w.ant_name for w in x.sync_info.on_wait
        )
    ]
    assert len(vestigial) == 5, [x.name for x in vestigial]
    for x in vestigial:
        end_blk.instructions.remove(x)

    nc.compile()
    return nc


def _pack31(bits):
    """bits [..., D] uint8 -> [..., WPK] int32 words, 31 payload bits per
    word at positions 0..29 and 31 (bit 30 always zero -> never NaN/Inf)."""
    lead = bits.shape[:-1]
    b32 = np.zeros(lead + (WPK, 32), dtype=np.uint8)
    pad = np.zeros(lead + (DPAD,), dtype=np.uint8)
    pad[..., :D] = bits
    pad = pad.reshape(lead + (WPK, PAYLOAD))
    b32[..., :30] = pad[..., :30]
    b32[..., 31] = pad[..., 30]
    words = np.packbits(b32.reshape(lead + (WPK * 32,)), axis=-1, bitorder="little")
    return words.reshape(lead + (WPK * 4,)).view(np.int32)


def _pack_inputs(x, bit_weights):
    x = np.asarray(x).astype(np.uint8)
    bw = np.ascontiguousarray(np.asarray(bit_weights).astype(np.uint8))
    notx = (1 - x).astype(np.uint8)
    nxp = _pack31(notx)  # [WPK]
    wp = _pack31(bw)  # [L, WPK]
    in_maps = []
    for i in range(NCORES):
        shard = wp[i * LSH : (i + 1) * LSH].reshape(128, NB, WPK)
        wx = np.empty((128, NB + 1, WPK), dtype=np.int32)
        wx[:, 0, :] = nxp
        wx[:, 1:, :] = shard
        in_maps.append({"wx": wx.reshape(128, (NB + 1) * WPK)})
    return in_maps


def _gather(results):
    outs = []
    for i in range(NCORES):
        # [1, 128, 1, NB] fp32 violation flags; res[0, p, 0, b] covers
        # neuron 8p + b; flag == 0.0 means no violated requirement -> True
        res = results[i]["res"].reshape(128, NB)
        outs.append(res.reshape(-1) == 0.0)
    return np.concatenate(outs).astype(np.bool_)


def _get_compiled():
    global _compiled
    if _compiled is None:
        _compiled = _build()
    return _compiled


def kernel(x, bit_weights):
    from concourse import bass_utils

    nc = _get_compiled()
    in_maps = _pack_inputs(x, bit_weights)
    r = bass_utils.run_bass_kernel_spmd(nc, in_maps, core_ids=list(range(NCORES)))
    return _gather(r.results)


# revision 50
# speedup vs baseline: 1.2904x; 1.0066x over previous
"""HardAndLayer on 8 Trainium2 NeuronCores.

out[l] = AND_d (x[d] OR NOT w[l,d])  ==  no d with (w[l,d] AND NOT x[d])

Strategy (per sharding hint): shard bit_weights row-wise (neuron dim) across
8 cores, x replicated, no collectives.

Wire format: bools are bit-packed 31 per int32 word with bit 30 forced
zero, so no word can form an fp32 NaN/Inf pattern (the DVE fp32 stream
path canonicalizes NaN operands — HW-verified failure with full 32-bit
packing). DRAM tensors are declared int32; on device the SBUF APs are
bitcast to fp32 for the custom DVE op, whose datapath is bitwise: per row
    acc[p] = fold_logical_or_j (w_packed[p, j] BITWISE_AND notx_packed[j])
(identity fp32 converter, BITWISE_AND preserves raw bits, LOGICAL_OR folds
on bit-pattern truthiness; -0.0/NaN/denormal patterns all count as nonzero).
out[l] = (acc == 0), applied on the host to the DMA'd per-neuron flags.

Pipeline: the weight shard streams in 4 HWDGE DMA chunks (each chunk->DVE
edge re-pays the ~900ns DMA-semaphore latency, so sizes are chosen to
keep the per-chunk anchors A_k = arrival_k + 900 + dve_work_after_k flat;
more chunks would serialize on the single 625ns/instr HWDGE). The result
write is a prepared SWDGE kv_writeback fired by trigger_dma right after
the last DVE accumulator write: its descriptors are generated early, off
the critical path, so the tail skips HWDGE descriptor-gen and the DGE
start delay entirely.

Layout: partition p of a core holds its 8 consecutive neuron rows
(8 KB contiguous per partition); res[p, b] = out[8p + b].
"""

import numpy as np

L = 8192
D = 8192
NCORES = 8
LSH = L // NCORES  # 1024 neuron rows per core
PAYLOAD = 31  # bits per packed word; bit 30 held zero -> never NaN/Inf.
# The DVE fp32 stream path canonicalizes NaN-pattern operands (HW-verified:
# 32-bit packing fails exactly on words with the exponent field all-ones),
# so the wire format must never form one. Denormals and -0.0 pass bit-exact.
WPK = -(-D // PAYLOAD)  # 265 packed words per neuron row
DPAD = WPK * PAYLOAD
NB = LSH // 128  # 8 neuron rows per partition
# Per-partition DRAM layout: [notx | row0 | ... | row7], 9*WPK words.
# Chunk sizes in row units (1 unit = WPK words = ~1 KiB/partition):
# (notx,r0,r1 | r2,r3 | r4,r5 | r6,r7) -- flat anchor schedule; 4 chunks
# keep the serial HWDGE descriptor-gen (625ns/instr) off the DMA stream.
CHUNK_UNITS = (3, 2, 2, 1, 1)
assert sum(CHUNK_UNITS) == NB + 1
CHUNK_COLS = tuple(u * WPK for u in CHUNK_UNITS)

_compiled = None
_custom_op = None


def _register_custom_op():
    """Register the fused AND+any op in the custom-DVE table (idempotent)."""
    global _custom_op
    if _custom_op is not None:
        return _custom_op
    from concourse import dve_ops
    from concourse.dve_spec import Spec, Src0, Src1, Zero, Bin, lower
    from concourse.dve_uop import AluOp, DveOpSpec

    name = "AND_ANY_ANT"
    for o in dve_ops.OPS:
        if o.name == name:
            _custom_op = o
            return o

    def _ref(in0, in1, c0, c1, c2):
        a = in0.view(np.uint32) & in1.view(np.uint32)
        acc = (
            (a.reshape(a.shape[0], -1) != 0)
            .any(axis=-1, keepdims=True)
            .astype(np.float32)
        )
        return a.view(np.float32), acc

    spec = Spec(
        body=Bin(AluOp.BITWISE_AND, Src0, Src1),
        accum=AluOp.LOGICAL_OR,
        accum_init=Zero,
        reference=_ref,
    )
    shas = {}
    for ver in ("v3", "v4"):
        try:
            uops = lower(spec, ver=ver)
            shas[ver] = DveOpSpec(name=name, uops=uops, rd1_en=True).sha(ver)
        except Exception:
            pass
    op = dve_ops.DveOp(name, spec, subdim=False, uops_sha=shas)
    dve_ops.OPS.append(op)
    dve_ops._SUB_OPCODE_FOR_NAME[name] = (
        dve_ops._CUSTOM_DVE_ROW_BASE + len(dve_ops.OPS) - 1
    )
    dve_ops.CUSTOM_DVE_SPECS[name] = spec
    _custom_op = op
    return op


def _build():
    import concourse.bacc as bacc
    import concourse.mybir as mybir
    from concourse import tile

    op = _register_custom_op()

    # Bass.__init__ registers 4 const APs via gpsimd.memset and emits a
    # kernel-start all-engine barrier. Nothing in this kernel reads the
    # const APs, and every cross-engine edge here is semaphore-gated (DMA
    # completion sems / engine ticks), so neither is needed; together they
    # delay the first DMA by ~700ns. Suppress both during construction.
    import concourse.bass as cbass

    orig_memset = cbass.BassGpSimd.memset
    orig_barrier = cbass.Bass.all_engine_barrier
    cbass.BassGpSimd.memset = lambda self, ap, constant: None
    cbass.Bass.all_engine_barrier = lambda self, *a, **k: None
    try:
        nc = bacc.Bacc(
            "TRN2",
            target_bir_lowering=False,
            debug=False,
            enable_asserts=False,
            num_devices=NCORES,
        )
    finally:
        cbass.BassGpSimd.memset = orig_memset
        cbass.Bass.all_engine_barrier = orig_barrier
    TOT = (NB + 1) * WPK
    wx = nc.dram_tensor("wx", [128, TOT], mybir.dt.int32, kind="ExternalInput")
    # kv_writeback-shaped result: [batch=1, dhi=128, dho=1, n_ctx=NB]
    res = nc.dram_tensor(
        "res", [1, 128, 1, NB], mybir.dt.float32, kind="ExternalOutput"
    )

    with tile.TileContext(nc) as tc:
        with (
            tc.tile_pool(name="wpool", bufs=1) as wpool,
            tc.tile_pool(name="small", bufs=1) as small,
        ):
            acc = small.tile([128, NB], mybir.dt.float32)
            idx = small.tile([128, 1], mybir.dt.int32)
            m = small.tile([128, WPK], mybir.dt.float32, tag="m")
            nc.gpsimd.memset(idx[:], 0)
            res_sem = nc.alloc_semaphore("res_dma_sem")

            # Weight + notx stream: HWDGE chunks, alternating SP/Act.
            tiles = []
            c0 = 0
            for ci, cw in enumerate(CHUNK_COLS):
                wt = wpool.tile([128, cw], mybir.dt.int32, tag=f"wt{ci}")
                dma_eng = nc.sync if ci % 2 == 0 else nc.scalar
                dma_eng.dma_start(wt[:], wx[:, c0 : c0 + cw])
                tiles.append((wt, c0, cw))
                c0 += cw

            def words_ap(col, n):
                """fp32-bitcast AP over words [col, col+n) of the stream."""
                for wt, tc0, tcw in tiles:
                    if tc0 <= col and col + n <= tc0 + tcw:
                        return wt[:, col - tc0 : col - tc0 + n].bitcast(
                            mybir.dt.float32
                        )
                raise AssertionError(f"span {col}+{n} crosses a chunk boundary")

            nx_ap = words_ap(0, WPK)  # notx is the stream head
            for gb in range(NB):
                nc.vector._custom_dve(
                    op,
                    out=m[:],
                    in0=words_ap((gb + 1) * WPK, WPK),
                    in1=nx_ap,
                    accum_out=acc[:, gb : gb + 1],
                )
            # Prepared result writeback: the prep only generates descriptors
            # (its RAW dep on `acc` is demoted to a no-sync edge, so the gen
            # runs early on the idle Pool engine); the transfer fires at
            # trigger_dma, which carries the sync deps on the accumulator
            # writes. The tail thus skips HWDGE desc-gen + DGE start delay.
            nc.gpsimd.kv_writeback(
                out_ap=res[:, :, :, :],
                in_ap=acc[:].rearrange("p (a b n) -> p a b n", a=1, b=1),
                ctx_idxs_ap=idx[:],
                prepare_only=True,
                sem=res_sem,
            )
            nc.gpsimd.trigger_dma(count=None)

    # Tile assigns each SWDGE prep a DMASW lane: consumers and the
    # end-of-kernel gather wait on DMASW<k> >= 16, but pass 2 leaves the
    # descriptor sem slot (on_update[0]) at our explicit `sem=`, which
    # nothing then waits on. Retarget each prep's descriptor sem to its
    # Tile-assigned lane semaphore (lanes are assigned to Pool DMA
    # instructions in program order) so the DMA completions tick the
    # lanes the waits actually watch.
    dmasw_ids = {}
    preps = []
    prep_blk = None
    for blk in nc.m.functions[0].blocks:
        for inst in blk.instructions:
            tn = type(inst).__name__
            if (
                tn in ("InstKVWritebackAnt", "InstDMAGatherAnt")
                and getattr(inst, "gen_mode", 0) == 1
            ):
                preps.append(inst)
                prep_blk = blk
            si = inst.sync_info
            if si is not None:
                for w in si.on_wait:
                    if w.ant_name and w.ant_name.startswith("DMASW"):
                        dmasw_ids[w.ant_name.split("_")[0]] = w.id
    assert len(preps) == 1 and dmasw_ids
    nlanes = len(dmasw_ids)
    for k, prep in enumerate(preps):
        upd = prep.sync_info.on_update
        assert upd[0].ant_name in ("nx_dma_sem", "res_dma_sem")
        upd[0].id = dmasw_ids[f"DMASW{k % nlanes}"]
        prep.sync_info.on_update = upd

    # Tile attaches the result-trigger's data dep (all DVE accumulator
    # writes) to the kv_writeback PREP, which would push the ~1us
    # descriptor-gen onto the post-compute critical path. Desc-gen reads
    # no tensor data, so move the DVE wait from the prep to the final
    # trigger: gen runs early on the idle Pool engine and only the
    # transfer launch gates on the DVE ops.
    kv_prep = preps[0]
    assert type(kv_prep).__name__ == "InstKVWritebackAnt"
    trig_inst = None
    for inst in prep_blk.instructions:
        if type(inst).__name__ == "InstTriggerDma":
            trig_inst = inst  # keep last
    prep_si = kv_prep.sync_info
    dve_waits = [
        w for w in prep_si.on_wait if w.ant_name and w.ant_name.startswith("DVE")
    ]
    assert len(dve_waits) == 1 and trig_inst is not None
    prep_si.on_wait = [
        w
        for w in prep_si.on_wait
        if not (w.ant_name and w.ant_name.startswith("DVE"))
    ]
    trig_si = trig_inst.sync_info
    trig_si.on_wait = list(trig_si.on_wait) + dve_waits

    # Strip DMA-completion waits that are already implied by same-engine
    # program order: the second DVE op of a 2-row chunk re-waits the same
    # semaphore threshold the first op already cleared.
    seen: set = set()
    for inst in prep_blk.instructions:
        if type(inst).__name__ != "InstCustomDveAnt":
            continue
        si = inst.sync_info
        if si is None:
            continue
        keep = []
        for w in si.on_wait:
            key = (w.id, w.wait_mode, w.wait_value)
            if key in seen:
                continue
            keep.append(w)
            seen.add(key)
        si.on_wait = keep

    # Hoist chunk 0's DMACopy ahead of SP's entry-block branch: with the
    # start barrier suppressed, SP's preamble is just register setup, and
    # the branch's ~50ns SEQ slot otherwise delays the first HWDGE
    # descriptor-gen. The copy has no waits and SP-stream order (and hence
    # HWDGE arbitration order) is unchanged.
    main_blk2 = next(b for b in nc.m.functions[0].blocks if b.name == "main")
    tile_blk = next(
        b
        for b in nc.m.functions[0].blocks
        if "tile_context" in b.name and not b.name.endswith("_end")
    )
    c0_dma = next(
        x
        for x in tile_blk.instructions
        if type(x).__name__ == "InstDMACopy" and str(x.engine).endswith("SP")
    )
    assert not (c0_dma.sync_info and list(c0_dma.sync_info.on_wait))
    tile_blk.instructions.remove(c0_dma)
    sp_branch_i = next(
        i
        for i, x in enumerate(main_blk2.instructions)
        if type(x).__name__ == "InstUnconditionalBranch"
        and str(x.engine).endswith("SP")
    )
    main_blk2.instructions.insert(sp_branch_i, c0_dma)

    # The epilogue runs TWO gather/release barrier rounds around the
    # semaphore range-clear. Round 2 is pure exit synchronization: every
    # engine already drained in round 1, the barrier sems are balanced at
    # zero after round 1, and program completion waits for all engines
    # regardless. Delete round 2 (everything after the clear), keeping one
    # Pool drain so the clear's engine op is flushed before Pool's stream
    # ends. The DMA-wait -> round-1 -> clear ordering that protects
    # cross-invocation semaphore state is untouched.
    end_blk = next(b for b in nc.m.functions[0].blocks if b.name.endswith("_end"))
    insts = end_blk.instructions
    clear_i = next(
        i
        for i, x in enumerate(insts)
        if getattr(x, "op_name", None) == "EVENT_SEMAPHORE_RANGE_CLEAR"
    )
    tail = list(insts[clear_i + 1 :])
    keep_drain = next(
        x
        for x in tail
        if type(x).__name__ == "InstDrain" and str(x.engine).endswith("Pool")
    )
    for x in tail:
        if x is not keep_drain:
            insts.remove(x)

    # The result-DMA completion wait sits on SP's end-drain, so after the
    # sem fires the path still runs SP-drain -> gather-inc -> sem-prop
    # before Pool (who must order the clear after DMA completion) can
    # proceed. Move the DMASW wait onto Pool's round-1 gather directly:
    # SP increments the gather early and Pool's gather carries both
    # conditions, deleting the SP hop from the critical path. Increment/
    # subtract arithmetic is unchanged -- only wait placement moves.
    sp_drain = next(
        x
        for x in end_blk.instructions
        if type(x).__name__ == "InstDrain"
        and str(x.engine).endswith("SP")
        and x.sync_info is not None
        and any(
            w.ant_name and w.ant_name.startswith("DMASW")
            for w in x.sync_info.on_wait
        )
    )
    pool_gather = next(
        x
        for x in end_blk.instructions
        if type(x).__name__ == "InstEventSemaphore"
        and str(x.engine).endswith("Pool")
        and x.sync_info is not None
        and any(
            w.ant_name and "gather" in w.ant_name for w in x.sync_info.on_wait
        )
    )
    sp_si = sp_drain.sync_info
    moved = [
        w for w in sp_si.on_wait if w.ant_name and w.ant_name.startswith("DMASW")
    ]
    assert len(moved) == 1
    sp_si.on_wait = [
        w
        for w in sp_si.on_wait
        if not (w.ant_name and w.ant_name.startswith("DMASW"))
    ]
    pg_si = pool_gather.sync_info
    pg_si.on_wait = list(pg_si.on_wait) + moved

    # With round 2 deleted, the release half of the round-1 handshake is
    # vestigial: engines would only pause at exit for Pool's release-add,
    # and the add sits serially on Pool's chain ahead of the clear.
    # Delete the release-add and the engines' release>=1 waits; the
    # release semaphore is then never touched (balanced at zero), and the
    # gather>=4 condition alone still orders the clear after all engines.
    vestigial = [
        x
        for x in list(end_blk.instructions)
        if type(x).__name__ == "InstEventSemaphore"
        and x.sync_info is not None
        and (
            any(
                w.ant_name and "release" in w.ant_name and w.wait_value
                for w in x.sync_info.on_wait
            )
            or any(
                u.ant_name and "release" in u.ant_name
                for u in x.sync_info.on_update
            )
        )
        and not any(
            w.ant_name and "gather" in w.ant_name for w in x.sync_info.on_wait
        )
    ]
    assert len(vestigial) == 5, [x.name for x in vestigial]
    for x in vestigial:
        end_blk.instructions.remove(x)

    nc.compile()
    return nc


def _pack31(bits):
    """bits [..., D] uint8 -> [..., WPK] int32 words, 31 payload bits per
    word at positions 0..29 and 31 (bit 30 always zero -> never NaN/Inf)."""
    lead = bits.shape[:-1]
    b32 = np.zeros(lead + (WPK, 32), dtype=np.uint8)
    pad = np.zeros(lead + (DPAD,), dtype=np.uint8)
    pad[..., :D] = bits
    pad = pad.reshape(lead + (WPK, PAYLOAD))
    b32[..., :30] = pad[..., :30]
    b32[..., 31] = pad[..., 30]
    words = np.packbits(b32.reshape(lead + (WPK * 32,)), axis=-1, bitorder="little")
    return words.reshape(lead + (WPK * 4,)).view(np.int32)


def _pack_inputs(x, bit_weights):
    x = np.asarray(x).astype(np.uint8)
    bw = np.ascontiguousarray(np.asarray(bit_weights).astype(np.uint8))
    notx = (1 - x).astype(np.uint8)
    nxp = _pack31(notx)  # [WPK]
    wp = _pack31(bw)  # [L, WPK]
    in_maps = []
    for i in range(NCORES):
        shard = wp[i * LSH : (i + 1) * LSH].reshape(128, NB, WPK)
        wx = np.empty((128, NB + 1, WPK), dtype=np.int32)
        wx[:, 0, :] = nxp
        wx[:, 1:, :] = shard
        in_maps.append({"wx": wx.reshape(128, (NB + 1) * WPK)})
    return in_maps


def _gather(results):
    outs = []
    for i in range(NCORES):
        # [1, 128, 1, NB] fp32 violation flags; res[0, p, 0, b] covers
        # neuron 8p + b; flag == 0.0 means no violated requirement -> True
        res = results[i]["res"].reshape(128, NB)
        outs.append(res.reshape(-1) == 0.0)
    return np.concatenate(outs).astype(np.bool_)


def _get_compiled():
    global _compiled
    if _compiled is None:
        _compiled = _build()
    return _compiled


def kernel(x, bit_weights):
    from concourse import bass_utils

    nc = _get_compiled()
    in_maps = _pack_inputs(x, bit_weights)
    r = bass_utils.run_bass_kernel_spmd(nc, in_maps, core_ids=list(range(NCORES)))
    return _gather(r.results)


# revision 51
# speedup vs baseline: 1.2966x; 1.0048x over previous
"""HardAndLayer on 8 Trainium2 NeuronCores.

out[l] = AND_d (x[d] OR NOT w[l,d])  ==  no d with (w[l,d] AND NOT x[d])

Strategy (per sharding hint): shard bit_weights row-wise (neuron dim) across
8 cores, x replicated, no collectives.

Wire format: bools are bit-packed 31 per int32 word with bit 30 forced
zero, so no word can form an fp32 NaN/Inf pattern (the DVE fp32 stream
path canonicalizes NaN operands — HW-verified failure with full 32-bit
packing). DRAM tensors are declared int32; on device the SBUF APs are
bitcast to fp32 for the custom DVE op, whose datapath is bitwise: per row
    acc[p] = fold_logical_or_j (w_packed[p, j] BITWISE_AND notx_packed[j])
(identity fp32 converter, BITWISE_AND preserves raw bits, LOGICAL_OR folds
on bit-pattern truthiness; -0.0/NaN/denormal patterns all count as nonzero).
out[l] = (acc == 0), applied on the host to the DMA'd per-neuron flags.

Pipeline: the weight shard streams in 4 HWDGE DMA chunks (each chunk->DVE
edge re-pays the ~900ns DMA-semaphore latency, so sizes are chosen to
keep the per-chunk anchors A_k = arrival_k + 900 + dve_work_after_k flat;
more chunks would serialize on the single 625ns/instr HWDGE). The result
write is a prepared SWDGE kv_writeback fired by trigger_dma right after
the last DVE accumulator write: its descriptors are generated early, off
the critical path, so the tail skips HWDGE descriptor-gen and the DGE
start delay entirely.

Layout: partition p of a core holds its 8 consecutive neuron rows
(8 KB contiguous per partition); res[p, b] = out[8p + b].
"""

import numpy as np

L = 8192
D = 8192
NCORES = 8
LSH = L // NCORES  # 1024 neuron rows per core
PAYLOAD = 31  # bits per packed word; bit 30 held zero -> never NaN/Inf.
# The DVE fp32 stream path canonicalizes NaN-pattern operands (HW-verified:
# 32-bit packing fails exactly on words with the exponent field all-ones),
# so the wire format must never form one. Denormals and -0.0 pass bit-exact.
WPK = -(-D // PAYLOAD)  # 265 packed words per neuron row
DPAD = WPK * PAYLOAD
NB = LSH // 128  # 8 neuron rows per partition
# Per-partition DRAM layout: [notx | row0 | ... | row7], 9*WPK words.
# Chunk sizes in row units (1 unit = WPK words = ~1 KiB/partition):
# (notx,r0,r1 | r2,r3 | r4,r5 | r6,r7) -- flat anchor schedule; 4 chunks
# keep the serial HWDGE descriptor-gen (625ns/instr) off the DMA stream.
CHUNK_UNITS = (3, 2, 2, 1, 1)
assert sum(CHUNK_UNITS) == NB + 1
CHUNK_COLS = tuple(u * WPK for u in CHUNK_UNITS)

_compiled = None
_custom_op = None


def _register_custom_op():
    """Register the fused AND+any op in the custom-DVE table (idempotent)."""
    global _custom_op
    if _custom_op is not None:
        return _custom_op
    from concourse import dve_ops
    from concourse.dve_spec import Spec, Src0, Src1, Zero, Bin, lower
    from concourse.dve_uop import AluOp, DveOpSpec

    name = "AND_ANY_ANT"
    for o in dve_ops.OPS:
        if o.name == name:
            _custom_op = o
            return o

    def _ref(in0, in1, c0, c1, c2):
        a = in0.view(np.uint32) & in1.view(np.uint32)
        acc = (
            (a.reshape(a.shape[0], -1) != 0)
            .any(axis=-1, keepdims=True)
            .astype(np.float32)
        )
        return a.view(np.float32), acc

    spec = Spec(
        body=Bin(AluOp.BITWISE_AND, Src0, Src1),
        accum=AluOp.LOGICAL_OR,
        accum_init=Zero,
        reference=_ref,
    )
    shas = {}
    for ver in ("v3", "v4"):
        try:
            uops = lower(spec, ver=ver)
            shas[ver] = DveOpSpec(name=name, uops=uops, rd1_en=True).sha(ver)
        except Exception:
            pass
    op = dve_ops.DveOp(name, spec, subdim=False, uops_sha=shas)
    dve_ops.OPS.append(op)
    dve_ops._SUB_OPCODE_FOR_NAME[name] = (
        dve_ops._CUSTOM_DVE_ROW_BASE + len(dve_ops.OPS) - 1
    )
    dve_ops.CUSTOM_DVE_SPECS[name] = spec
    _custom_op = op
    return op


def _build():
    import concourse.bacc as bacc
    import concourse.mybir as mybir
    from concourse import tile

    op = _register_custom_op()

    # Bass.__init__ registers 4 const APs via gpsimd.memset and emits a
    # kernel-start all-engine barrier. Nothing in this kernel reads the
    # const APs, and every cross-engine edge here is semaphore-gated (DMA
    # completion sems / engine ticks), so neither is needed; together they
    # delay the first DMA by ~700ns. Suppress both during construction.
    import concourse.bass as cbass

    orig_memset = cbass.BassGpSimd.memset
    orig_barrier = cbass.Bass.all_engine_barrier
    cbass.BassGpSimd.memset = lambda self, ap, constant: None
    cbass.Bass.all_engine_barrier = lambda self, *a, **k: None
    try:
        nc = bacc.Bacc(
            "TRN2",
            target_bir_lowering=False,
            debug=False,
            enable_asserts=False,
            num_devices=NCORES,
        )
    finally:
        cbass.BassGpSimd.memset = orig_memset
        cbass.Bass.all_engine_barrier = orig_barrier
    TOT = (NB + 1) * WPK
    wx = nc.dram_tensor("wx", [128, TOT], mybir.dt.int32, kind="ExternalInput")
    # kv_writeback-shaped result: [batch=1, dhi=128, dho=1, n_ctx=NB]
    res = nc.dram_tensor(
        "res", [1, 128, 1, NB], mybir.dt.float32, kind="ExternalOutput"
    )

    with tile.TileContext(nc) as tc:
        with (
            tc.tile_pool(name="wpool", bufs=1) as wpool,
            tc.tile_pool(name="small", bufs=1) as small,
        ):
            acc = small.tile([128, NB], mybir.dt.float32)
            idx = small.tile([128, 1], mybir.dt.int32)
            m = small.tile([128, WPK], mybir.dt.float32, tag="m")
            nc.gpsimd.memset(idx[:], 0)
            res_sem = nc.alloc_semaphore("res_dma_sem")

            # Weight + notx stream: HWDGE chunks, alternating SP/Act.
            tiles = []
            c0 = 0
            for ci, cw in enumerate(CHUNK_COLS):
                wt = wpool.tile([128, cw], mybir.dt.int32, tag=f"wt{ci}")
                dma_eng = nc.sync if ci % 2 == 0 else nc.scalar
                dma_eng.dma_start(wt[:], wx[:, c0 : c0 + cw])
                tiles.append((wt, c0, cw))
                c0 += cw

            def words_ap(col, n):
                """fp32-bitcast AP over words [col, col+n) of the stream."""
                for wt, tc0, tcw in tiles:
                    if tc0 <= col and col + n <= tc0 + tcw:
                        return wt[:, col - tc0 : col - tc0 + n].bitcast(
                            mybir.dt.float32
                        )
                raise AssertionError(f"span {col}+{n} crosses a chunk boundary")

            nx_ap = words_ap(0, WPK)  # notx is the stream head
            for gb in range(NB):
                nc.vector._custom_dve(
                    op,
                    out=m[:],
                    in0=words_ap((gb + 1) * WPK, WPK),
                    in1=nx_ap,
                    accum_out=acc[:, gb : gb + 1],
                )
            # Prepared result writeback: the prep only generates descriptors
            # (its RAW dep on `acc` is demoted to a no-sync edge, so the gen
            # runs early on the idle Pool engine); the transfer fires at
            # trigger_dma, which carries the sync deps on the accumulator
            # writes. The tail thus skips HWDGE desc-gen + DGE start delay.
            nc.gpsimd.kv_writeback(
                out_ap=res[:, :, :, :],
                in_ap=acc[:].rearrange("p (a b n) -> p a b n", a=1, b=1),
                ctx_idxs_ap=idx[:],
                prepare_only=True,
                sem=res_sem,
            )
            nc.gpsimd.trigger_dma(count=None)

    # Tile assigns each SWDGE prep a DMASW lane: consumers and the
    # end-of-kernel gather wait on DMASW<k> >= 16, but pass 2 leaves the
    # descriptor sem slot (on_update[0]) at our explicit `sem=`, which
    # nothing then waits on. Retarget each prep's descriptor sem to its
    # Tile-assigned lane semaphore (lanes are assigned to Pool DMA
    # instructions in program order) so the DMA completions tick the
    # lanes the waits actually watch.
    dmasw_ids = {}
    preps = []
    prep_blk = None
    for blk in nc.m.functions[0].blocks:
        for inst in blk.instructions:
            tn = type(inst).__name__
            if (
                tn in ("InstKVWritebackAnt", "InstDMAGatherAnt")
                and getattr(inst, "gen_mode", 0) == 1
            ):
                preps.append(inst)
                prep_blk = blk
            si = inst.sync_info
            if si is not None:
                for w in si.on_wait:
                    if w.ant_name and w.ant_name.startswith("DMASW"):
                        dmasw_ids[w.ant_name.split("_")[0]] = w.id
    assert len(preps) == 1 and dmasw_ids
    nlanes = len(dmasw_ids)
    for k, prep in enumerate(preps):
        upd = prep.sync_info.on_update
        assert upd[0].ant_name in ("nx_dma_sem", "res_dma_sem")
        upd[0].id = dmasw_ids[f"DMASW{k % nlanes}"]
        prep.sync_info.on_update = upd

    # Tile attaches the result-trigger's data dep (all DVE accumulator
    # writes) to the kv_writeback PREP, which would push the ~1us
    # descriptor-gen onto the post-compute critical path. Desc-gen reads
    # no tensor data, so move the DVE wait from the prep to the final
    # trigger: gen runs early on the idle Pool engine and only the
    # transfer launch gates on the DVE ops.
    kv_prep = preps[0]
    assert type(kv_prep).__name__ == "InstKVWritebackAnt"
    trig_inst = None
    for inst in prep_blk.instructions:
        if type(inst).__name__ == "InstTriggerDma":
            trig_inst = inst  # keep last
    prep_si = kv_prep.sync_info
    dve_waits = [
        w for w in prep_si.on_wait if w.ant_name and w.ant_name.startswith("DVE")
    ]
    assert len(dve_waits) == 1 and trig_inst is not None
    prep_si.on_wait = [
        w
        for w in prep_si.on_wait
        if not (w.ant_name and w.ant_name.startswith("DVE"))
    ]
    trig_si = trig_inst.sync_info
    trig_si.on_wait = list(trig_si.on_wait) + dve_waits

    # Strip DMA-completion waits that are already implied by same-engine
    # program order: the second DVE op of a 2-row chunk re-waits the same
    # semaphore threshold the first op already cleared.
    seen: set = set()
    for inst in prep_blk.instructions:
        if type(inst).__name__ != "InstCustomDveAnt":
            continue
        si = inst.sync_info
        if si is None:
            continue
        keep = []
        for w in si.on_wait:
            key = (w.id, w.wait_mode, w.wait_value)
            if key in seen:
                continue
            keep.append(w)
            seen.add(key)
        si.on_wait = keep

    # Hoist chunk 0's DMACopy ahead of SP's entry-block branch: with the
    # start barrier suppressed, SP's preamble is just register setup, and
    # the branch's ~50ns SEQ slot otherwise delays the first HWDGE
    # descriptor-gen. The copy has no waits and SP-stream order (and hence
    # HWDGE arbitration order) is unchanged.
    main_blk2 = next(b for b in nc.m.functions[0].blocks if b.name == "main")
    tile_blk = next(
        b
        for b in nc.m.functions[0].blocks
        if "tile_context" in b.name and not b.name.endswith("_end")
    )
    c0_dma = next(
        x
        for x in tile_blk.instructions
        if type(x).__name__ == "InstDMACopy" and str(x.engine).endswith("SP")
    )
    assert not (c0_dma.sync_info and list(c0_dma.sync_info.on_wait))
    tile_blk.instructions.remove(c0_dma)
    sp_branch_i = next(
        i
        for i, x in enumerate(main_blk2.instructions)
        if type(x).__name__ == "InstUnconditionalBranch"
        and str(x.engine).endswith("SP")
    )
    main_blk2.instructions.insert(sp_branch_i, c0_dma)

    # The epilogue runs TWO gather/release barrier rounds around the
    # semaphore range-clear. Round 2 is pure exit synchronization: every
    # engine already drained in round 1, the barrier sems are balanced at
    # zero after round 1, and program completion waits for all engines
    # regardless. Delete round 2 (everything after the clear), keeping one
    # Pool drain so the clear's engine op is flushed before Pool's stream
    # ends. The DMA-wait -> round-1 -> clear ordering that protects
    # cross-invocation semaphore state is untouched.
    end_blk = next(b for b in nc.m.functions[0].blocks if b.name.endswith("_end"))
    insts = end_blk.instructions
    clear_i = next(
        i
        for i, x in enumerate(insts)
        if getattr(x, "op_name", None) == "EVENT_SEMAPHORE_RANGE_CLEAR"
    )
    tail = list(insts[clear_i + 1 :])
    keep_drain = next(
        x
        for x in tail
        if type(x).__name__ == "InstDrain" and str(x.engine).endswith("Pool")
    )
    for x in tail:
        if x is not keep_drain:
            insts.remove(x)

    # The result-DMA completion wait sits on SP's end-drain, so after the
    # sem fires the path still runs SP-drain -> gather-inc -> sem-prop
    # before Pool (who must order the clear after DMA completion) can
    # proceed. Move the DMASW wait onto Pool's round-1 gather directly:
    # SP increments the gather early and Pool's gather carries both
    # conditions, deleting the SP hop from the critical path. Increment/
    # subtract arithmetic is unchanged -- only wait placement moves.
    sp_drain = next(
        x
        for x in end_blk.instructions
        if type(x).__name__ == "InstDrain"
        and str(x.engine).endswith("SP")
        and x.sync_info is not None
        and any(
            w.ant_name and w.ant_name.startswith("DMASW")
            for w in x.sync_info.on_wait
        )
    )
    pool_gather = next(
        x
        for x in end_blk.instructions
        if type(x).__name__ == "InstEventSemaphore"
        and str(x.engine).endswith("Pool")
        and x.sync_info is not None
        and any(
            w.ant_name and "gather" in w.ant_name for w in x.sync_info.on_wait
        )
    )
    sp_si = sp_drain.sync_info
    moved = [
        w for w in sp_si.on_wait if w.ant_name and w.ant_name.startswith("DMASW")
    ]
    assert len(moved) == 1
    sp_si.on_wait = [
        w
        for w in sp_si.on_wait
        if not (w.ant_name and w.ant_name.startswith("DMASW"))
    ]
    pg_si = pool_gather.sync_info
    pg_si.on_wait = list(pg_si.on_wait) + moved

    # With round 2 deleted, the release half of the round-1 handshake is
    # vestigial: engines would only pause at exit for Pool's release-add,
    # and the add sits serially on Pool's chain ahead of the clear.
    # Delete the release-add and the engines' release>=1 waits; the
    # release semaphore is then never touched (balanced at zero), and the
    # gather>=4 condition alone still orders the clear after all engines.
    vestigial = [
        x
        for x in list(end_blk.instructions)
        if type(x).__name__ == "InstEventSemaphore"
        and x.sync_info is not None
        and (
            any(
                w.ant_name and "release" in w.ant_name and w.wait_value
                for w in x.sync_info.on_wait
            )
            or any(
                u.ant_name and "release" in u.ant_name
                for u in x.sync_info.on_update
            )
        )
        and not any(
            w.ant_name and "gather" in w.ant_name for w in x.sync_info.on_wait
        )
    ]
    assert len(vestigial) == 5, [x.name for x in vestigial]
    for x in vestigial:
        end_blk.instructions.remove(x)

    # Pool's engine-flush drain sits between the gather (which now carries
    # the 900ns result-DMA wait) and the clear, but Pool's engine has been
    # idle since the prep's desc-gen finished ~3us earlier -- run the drain
    # BEFORE the gather so its SEQ slot comes off the post-sem chain.
    pool_seq = [
        (i, x)
        for i, x in enumerate(end_blk.instructions)
        if str(getattr(x, "engine", "")).endswith("Pool")
    ]
    g_pos = next(i for i, (_, x) in enumerate(pool_seq) if x is pool_gather)
    d_pos = next(
        i
        for i, (_, x) in enumerate(pool_seq)
        if i > g_pos and type(x).__name__ == "InstDrain"
    )
    idxs = [pool_seq[g_pos][0], pool_seq[d_pos][0]]
    drain_inst = pool_seq[d_pos][1]
    end_blk.instructions[idxs[0]] = drain_inst
    end_blk.instructions[idxs[1]] = pool_gather

    nc.compile()
    return nc


def _pack31(bits):
    """bits [..., D] uint8 -> [..., WPK] int32 words, 31 payload bits per
    word at positions 0..29 and 31 (bit 30 always zero -> never NaN/Inf)."""
    lead = bits.shape[:-1]
    b32 = np.zeros(lead + (WPK, 32), dtype=np.uint8)
    pad = np.zeros(lead + (DPAD,), dtype=np.uint8)
    pad[..., :D] = bits
    pad = pad.reshape(lead + (WPK, PAYLOAD))
    b32[..., :30] = pad[..., :30]
    b32[..., 31] = pad[..., 30]
    words = np.packbits(b32.reshape(lead + (WPK * 32,)), axis=-1, bitorder="little")
    return words.reshape(lead + (WPK * 4,)).view(np.int32)


def _pack_inputs(x, bit_weights):
    x = np.asarray(x).astype(np.uint8)
    bw = np.ascontiguousarray(np.asarray(bit_weights).astype(np.uint8))
    notx = (1 - x).astype(np.uint8)
    nxp = _pack31(notx)  # [WPK]
    wp = _pack31(bw)  # [L, WPK]
    in_maps = []
    for i in range(NCORES):
        shard = wp[i * LSH : (i + 1) * LSH].reshape(128, NB, WPK)
        wx = np.empty((128, NB + 1, WPK), dtype=np.int32)
        wx[:, 0, :] = nxp
        wx[:, 1:, :] = shard
        in_maps.append({"wx": wx.reshape(128, (NB + 1) * WPK)})
    return in_maps


def _gather(results):
    outs = []
    for i in range(NCORES):
        # [1, 128, 1, NB] fp32 violation flags; res[0, p, 0, b] covers
        # neuron 8p + b; flag == 0.0 means no violated requirement -> True
        res = results[i]["res"].reshape(128, NB)
        outs.append(res.reshape(-1) == 0.0)
    return np.concatenate(outs).astype(np.bool_)


def _get_compiled():
    global _compiled
    if _compiled is None:
        _compiled = _build()
    return _compiled


def kernel(x, bit_weights):
    from concourse import bass_utils

    nc = _get_compiled()
    in_maps = _pack_inputs(x, bit_weights)
    r = bass_utils.run_bass_kernel_spmd(nc, in_maps, core_ids=list(range(NCORES)))
    return _gather(r.results)
